# revision 1
# baseline (speedup 1.0000x reference)
"""Trainium2 Bass kernel for BlockFFTDirectPrior.

Computes out = irfft(einsum('bjn,ijn->bin', rfft(x_blocks), conj(W)))
reshaped to [B, 4096], for x [4096, 4096] f32, W [16, 16, 129] complex
(block size 256).

Strategy: data-parallel over the batch axis across 8 NeuronCores (512 rows
each); W-derived constants replicated. Per core, four PE stages:

  T: transpose x tiles (PE transpose vs identity)      -> xt [t, b] per block
  F: real DFT as matmul (contract t, K=2x128 chunks)   -> X  [n, b] per block
       R0 rows n=0..127 hold Xr[n]; R1 row 0 holds Xr[128] (Nyquist),
       rows p=1..127 hold Xi[p].
  E: per-frequency 16x16 complex mixing as 8-frequency block-diagonal
     matmuls (K = (j,f) = 128)                         -> Y [(i,f), b] per group
  I: real inverse DFT with the data as the stationary operand, which
     restores the [b, m] orientation for free            -> out [b, i*256+m]

All matmuls use float32r (TRN2's single-pass fp32 PE mode, 4x the fp32
rate; ~2.5e-4 rel error vs ~3e-7 for 2-pass fp32). DFT/IDFT row order is
swizzled to r = f*16+g so that the two partition regroupings between F/E
and E/I become plain affine SBUF->SBUF DMAs (partition dim leading), split
across the HWDGE (sync) and SWDGE (gpsimd) rings, which drive disjoint
8-SDMA-engine groups.
"""

import os
import numpy as np
from contextlib import ExitStack

import concourse.bass as bass
import concourse.tile as tile
from concourse import bacc, mybir
from concourse.bass_utils import run_bass_kernel_spmd

NCORES = 8
B_FULL, D_IN, D_OUT, BS = 4096, 4096, 4096, 256
BC = B_FULL // NCORES          # 512 batch rows per core
KIN = KOUT = 16
NG = 16                        # groups of 8 frequencies covering n=0..127
F32 = mybir.dt.float32
F32R = mybir.dt.float32r       # single-pass PE fp32 mode (4x faster matmul)

_CACHE = {}
LAST_RESULTS = None            # BassKernelResults of the most recent run


# DFT/IDFT row swizzle: row r = f*16+g holds frequency n = 8g+f. This makes
# both partition regroups plain affine DMAs (partition dim outermost, step 1).
PERM = np.array([8 * (r % 16) + r // 16 for r in range(128)])


def _build_consts(W_real, W_imag):
    """Constant matrices in the exact SBUF layouts the kernel reads."""
    f32 = np.float32
    t = np.arange(BS)
    n0 = np.arange(128)
    ang = 2.0 * np.pi / BS

    CF0 = np.cos(ang * np.outer(t, n0))
    CF1 = np.empty((BS, 128))
    CF1[:, 0] = np.cos(np.pi * t)
    p = np.arange(1, 128)
    CF1[:, 1:] = -np.sin(ang * np.outer(t, p))
    CF0 = CF0[:, PERM]
    CF1 = CF1[:, PERM]
    cfs = np.stack([
        np.concatenate([CF0[:128], CF0[128:]], axis=1),
        np.concatenate([CF1[:128], CF1[128:]], axis=1),
    ], axis=1).astype(f32)                                  # [128, 2, 256]

    # wpk[(f*16+j), g, c, (f*16+i)] = M_c[i, j, 8g+f];  M = (Wr, Wi, -Wi)
    wpk = np.zeros((128, NG, 3, 128), dtype=f32)
    jj = np.arange(KIN)[:, None, None]
    ii = np.arange(KOUT)[None, :, None]
    ff = np.arange(8)[None, None, :]
    for g in range(NG):
        for c, M in enumerate((W_real, W_imag, -W_imag)):
            wpk[ff * 16 + jj, g, c, ff * 16 + ii] = M[ii, jj, 8 * g + ff]
    wnyq = np.ascontiguousarray(W_real[:, :, 128].T).astype(f32)  # [j, i]

    m = np.arange(BS)
    D0 = np.empty((128, BS))
    D0[0] = 1.0 / BS
    nn = np.arange(1, 128)
    D0[1:] = (2.0 / BS) * np.cos(ang * np.outer(nn, m))
    D1 = np.empty((128, BS))
    D1[0] = ((-1.0) ** m) / BS
    D1[1:] = -(2.0 / BS) * np.sin(ang * np.outer(nn, m))
    dmat = np.stack([D0[PERM], D1[PERM]], axis=1).astype(f32)  # [128, 2, 256]

    ident = np.eye(128, dtype=f32)
    return {"cfs": cfs, "wpk": wpk, "wnyq": wnyq, "dmat": dmat, "ident": ident}


def _build_program():
    nc = bacc.Bacc(
        "TRN2", target_bir_lowering=False, debug=False, num_devices=NCORES
    )
    x_d = nc.dram_tensor("x", [BC, D_IN], F32, kind="ExternalInput").ap()
    cfs_d = nc.dram_tensor("cfs", [128, 2, 256], F32R, kind="ExternalInput").ap()
    wpk_d = nc.dram_tensor("wpk", [128, NG, 3, 128], F32R, kind="ExternalInput").ap()
    wnyq_d = nc.dram_tensor("wnyq", [KIN, KOUT], F32R, kind="ExternalInput").ap()
    dmat_d = nc.dram_tensor("dmat", [128, 2, 256], F32R, kind="ExternalInput").ap()
    ident_d = nc.dram_tensor("ident", [128, 128], F32, kind="ExternalInput").ap()
    out_d = nc.dram_tensor("out", [BC, D_OUT], F32, kind="ExternalOutput").ap()

    cp_state = [0]

    with tile.TileContext(nc) as tc, ExitStack() as ctx:
        def copy(dst, src):
            # alternate PSUM->SBUF copies between DVE and ACT
            if cp_state[0] % 2 == 0:
                nc.vector.tensor_copy(dst, src)
            else:
                nc.scalar.copy(dst, src)
            cp_state[0] += 1

        consts = ctx.enter_context(tc.tile_pool(name="consts", bufs=1))
        stg = ctx.enter_context(tc.tile_pool(name="stg", bufs=5))
        ps = ctx.enter_context(tc.tile_pool(name="ps", bufs=6, space="PSUM"))

        cfs = consts.tile([128, 2, 256], F32R)
        wpk = consts.tile([128, NG, 3, 128], F32R)
        wnyq = consts.tile([KIN, KOUT], F32R)
        dmat = consts.tile([128, 2, 256], F32R)
        ident = consts.tile([128, 128], F32)
        gnyq = consts.tile([KIN, BC], F32R)

        nc.sync.dma_start(cfs[:], cfs_d)
        nc.sync.dma_start(wpk[:], wpk_d)
        nc.sync.dma_start(wnyq[:], wnyq_d)
        nc.sync.dma_start(dmat[:], dmat_d)
        nc.sync.dma_start(ident[:], ident_d)

        # ---- load x: [b, d] in 4 chunks of 128 rows
        xs0 = stg.tile([128, 2, D_IN], F32, tag="stg")
        xs1 = stg.tile([128, 2, D_IN], F32, tag="stg")
        xsv = [xs0, xs1]
        for b4 in range(4):
            nc.sync.dma_start(
                xsv[b4 // 2][:, b4 % 2, :], x_d[128 * b4:128 * (b4 + 1), :]
            )

        # ---- stage T: xt[dc][t_lo, b], dc = j*2 + tc
        xt0 = stg.tile([128, 16, BC], F32R, tag="stg")
        xt1 = stg.tile([128, 16, BC], F32R, tag="stg")
        xtv = [xt0, xt1]
        for bc in range(4):
            for dcg in range(8):
                pt = ps.tile([128, 4, 128], F32, tag="ps")
                for q in range(4):
                    dc = dcg * 4 + q
                    nc.tensor.transpose(
                        pt[:, q, :],
                        xsv[bc // 2][:, bc % 2, 128 * dc:128 * (dc + 1)],
                        ident[:],
                    )
                dst = xtv[dcg // 4][
                    :, 4 * (dcg % 4):4 * (dcg % 4) + 4, 128 * bc:128 * (bc + 1)
                ]
                copy(dst, pt[:])

        # ---- stage F: real DFT (fp32r matmuls)
        xfr = stg.tile([128, KIN, BC], F32R, tag="stg")
        xfi = stg.tile([128, KIN, BC], F32R, tag="stg")
        ggr = stg.tile([128, NG, BC], F32R, tag="stg")
        ggi = stg.tile([128, NG, BC], F32R, tag="stg")
        for which, dstT in ((0, xfr), (1, xfi)):
            for j in range(KIN):
                pf = ps.tile([128, BC], F32, tag="ps")
                for tc_ in range(2):
                    nc.tensor.matmul(
                        pf[:],
                        cfs[:, which, 128 * tc_:128 * (tc_ + 1)],
                        xtv[j // 8][:, 2 * (j % 8) + tc_, :],
                        start=(tc_ == 0),
                        stop=(tc_ == 1),
                    )
                copy(dstT[:, j, :], pf[:])
            # regroup1 part for this half, split across both DMA rings so it
            # starts draining while the other half still computes:
            # gg*[(f*16+j), g, b] = xf*[f*16+g, j, b]
            dstG = ggr if which == 0 else ggi
            srcG = xfr if which == 0 else xfi
            for g in range(NG):
                eng = nc.sync if g % 2 == 0 else nc.gpsimd
                eng.dma_start(out=dstG[:, g, :], in_=srcG[g::16, :, :])
        nc.scalar.dma_start(out=gnyq[:], in_=xfi[0:1, :, :])

        # ---- stage E: blockdiag einsum (fp32r)
        yyr = stg.tile([128, NG, BC], F32R, tag="stg")
        yyi = stg.tile([128, NG, BC], F32R, tag="stg")
        yh0 = stg.tile([128, KOUT, BC], F32R, tag="stg")
        yh1 = stg.tile([128, KOUT, BC], F32R, tag="stg")
        for g in range(NG):
            pyr = ps.tile([128, BC], F32, tag="ps")
            nc.tensor.matmul(pyr[:], wpk[:, g, 0, :],
                             ggr[:, g, :], start=True, stop=False)
            nc.tensor.matmul(pyr[:], wpk[:, g, 1, :],
                             ggi[:, g, :], start=False, stop=True)
            copy(yyr[:, g, :], pyr[:])
        # regroup2-r starts while the yi half of the einsum still runs
        for i in range(KOUT):
            eng = nc.sync if i % 2 == 0 else nc.gpsimd
            eng.dma_start(out=yh0[:, i, :], in_=yyr[i::16, :, :])
        for g in range(NG):
            pyi = ps.tile([128, BC], F32, tag="ps")
            nc.tensor.matmul(pyi[:], wpk[:, g, 0, :],
                             ggi[:, g, :], start=True, stop=False)
            nc.tensor.matmul(pyi[:], wpk[:, g, 2, :],
                             ggr[:, g, :], start=False, stop=True)
            copy(yyi[:, g, :], pyi[:])
        # Nyquist einsum lands in the (f=0,g=0) rows of yyi (the otherwise
        # meaningless Zi[0] slots); regroup2 then routes it to yh1 row 0.
        pyn = ps.tile([KIN, BC], F32, tag="ps")
        nc.tensor.matmul(pyn[:], wnyq[:],
                         gnyq[:], start=True, stop=True)
        copy(yyi[0:KIN, 0, :], pyn[:])
        # ---- regroup2: yh0[f*16+g, i, b] = yyr[f*16+i, g, b]
        os0 = stg.tile([128, 2, D_OUT], F32, tag="stg")
        os1 = stg.tile([128, 2, D_OUT], F32, tag="stg")
        osv = [os0, os1]
        for i in range(KOUT):
            eng = nc.gpsimd if i % 2 == 0 else nc.sync
            eng.dma_start(out=yh1[:, i, :], in_=yyi[i::16, :, :])
        # ---- stage I: inverse DFT, data as stationary operand -> [b, m]
        for bs in range(4):
            for i in range(KOUT):
                po = ps.tile([128, BS], F32, tag="ps")
                nc.tensor.matmul(
                    po[:], yh0[:, i, 128 * bs:128 * (bs + 1)],
                    dmat[:, 0, :], start=True, stop=False)
                nc.tensor.matmul(
                    po[:], yh1[:, i, 128 * bs:128 * (bs + 1)],
                    dmat[:, 1, :], start=False, stop=True)
                copy(osv[bs // 2][:, bs % 2, BS * i:BS * (i + 1)], po[:])
            nc.sync.dma_start(
                out_d[128 * bs:128 * (bs + 1), :], osv[bs // 2][:, bs % 2, :]
            )

    nc.compile()
    return nc


def _get_program():
    if "nc" not in _CACHE:
        _CACHE["nc"] = _build_program()
    return _CACHE["nc"]


def _install_ntff_hook():
    """Provide antenv.axon_hooks (absent in this image) so that
    run_bass_kernel_spmd(trace=True) can capture NTFF profiles through the
    axon client library."""
    import sys
    import types
    import ctypes
    import contextlib

    if "antenv.axon_hooks" in sys.modules:
        return
    try:
        lib = ctypes.CDLL("/opt/axon/libaxon_pjrt.so")
    except OSError:
        return
    if not hasattr(lib, "axon_start_nrt_profile"):
        return
    lib.axon_start_nrt_profile.argtypes = [
        ctypes.POINTER(ctypes.c_int64),
        ctypes.c_size_t,
    ]
    lib.axon_start_nrt_profile.restype = ctypes.c_int64
    lib.axon_stop_nrt_profile.argtypes = [ctypes.c_char_p]
    lib.axon_stop_nrt_profile.restype = ctypes.c_int64

    @contextlib.contextmanager
    def _hook(output_dir, device_ids):
        import jax

        jax.devices()
        if device_ids:
            ids = (ctypes.c_int64 * len(device_ids))(*device_ids)
            rc = lib.axon_start_nrt_profile(ids, len(device_ids))
        else:
            rc = lib.axon_start_nrt_profile(None, 0)
        if rc != 0:
            raise RuntimeError(f"axon_start_nrt_profile rc={rc}")
        try:
            yield
        finally:
            n = lib.axon_stop_nrt_profile(str(output_dir).encode())
            print(f"ntff profile: {n} file(s) -> {output_dir}")

    mod = types.ModuleType("antenv.axon_hooks")
    state = {"hook": _hook}
    mod.get_axon_ntff_profile_hook = lambda: state["hook"]
    mod.set_axon_ntff_profile_hook = lambda h: state.update(hook=h)
    sys.modules["antenv.axon_hooks"] = mod
    import antenv

    antenv.axon_hooks = mod


def kernel(x, W_real, W_imag, block_size, out_features):
    global LAST_RESULTS
    x = np.ascontiguousarray(np.asarray(x, dtype=np.float32))
    Wr = np.asarray(W_real, dtype=np.float32)
    Wi = np.asarray(W_imag, dtype=np.float32)
    assert int(block_size) == BS and int(out_features) == D_OUT
    assert x.shape == (B_FULL, D_IN) and Wr.shape == (KOUT, KIN, 129)

    nc = _get_program()
    consts = _build_consts(Wr, Wi)
    core_ids = list(range(NCORES))
    in_maps = [
        {"x": np.ascontiguousarray(x[c * BC:(c + 1) * BC]), **consts}
        for c in core_ids
    ]
    trace = bool(int(os.environ.get("KERNEL_TRACE", "0")))
    if trace:
        _install_ntff_hook()
    res = run_bass_kernel_spmd(nc, in_maps, core_ids, trace=trace)
    LAST_RESULTS = res
    out = np.concatenate([res.results[c]["out"] for c in core_ids], axis=0)
    return np.ascontiguousarray(out.astype(np.float32))



# revision 7
# speedup vs baseline: 1.0479x; 1.0479x over previous
"""Trainium2 Bass kernel for BlockFFTDirectPrior.

Computes out = irfft(einsum('bjn,ijn->bin', rfft(x_blocks), conj(W)))
reshaped to [B, 4096], for x [4096, 4096] f32, W [16, 16, 129] complex
(block size 256).

Strategy: data-parallel over the batch axis across 8 NeuronCores (512 rows
each). Per core, the 512 rows are processed as two 256-row slabs flowing
through a 4-stage PE pipeline so input DMA, compute, the two partition
regroups, and output stores all overlap:

  T: transpose x tiles (PE transpose vs identity)     -> xt [t, dc, b] fp16
  F: real DFT as fp16 matmuls (contract t)            -> xf [n, j, ri, b]
  E: per-frequency 16x16 complex mixing as 8-frequency
     block-diagonal fp16 matmuls (K = (f,j) = 128)    -> yy [n', g, ri, b]
  I: real inverse DFT, data stationary (fp16 weights,
     FWL), which restores [b, m] orientation for free -> out rows

DFT/IDFT row order is swizzled to r = f*16+g so the two partition
regroups between F/E and E/I become per-g (resp. per-i) affine
SBUF->SBUF DMAs that carry both the real and imag halves in one
transfer. Regroups and stores ride the two fast HWDGE rings (sync +
scalar) ordered to match the pipeline; intermediates are fp16, which
halves regroup bytes and doubles LDWEIGHTS rate (FWL). Accumulation
groups are bank-interleaved in PSUM so one LDWEIGHTS feeds two matmuls.
"""

import os
import numpy as np
from contextlib import ExitStack

import concourse.bass as bass
import concourse.tile as tile
from concourse import bacc, mybir
from concourse.bass_utils import run_bass_kernel_spmd

NCORES = 8
B_FULL, D_IN, D_OUT, BS = 4096, 4096, 4096, 256
BC = B_FULL // NCORES          # 512 batch rows per core
SLAB = 256                     # rows per pipeline slab (2 slabs per core)
KIN = KOUT = 16
NG = 16                        # groups of 8 frequencies covering n=0..127
F16 = mybir.dt.float16
F32 = mybir.dt.float32

_CACHE = {}
LAST_RESULTS = None            # BassKernelResults of the most recent run


# DFT/IDFT row swizzle: row r = f*16+g holds frequency n = 8g+f. This makes
# both partition regroups plain affine DMAs (partition dim outermost).
PERM = np.array([8 * (r % 16) + r // 16 for r in range(128)])


def _build_consts(W_real, W_imag):
    """Constant matrices in the exact SBUF layouts the kernel reads."""
    f16 = np.float16
    t = np.arange(BS)
    n0 = np.arange(128)
    ang = 2.0 * np.pi / BS

    CF0 = np.cos(ang * np.outer(t, n0))
    CF1 = np.empty((BS, 128))
    CF1[:, 0] = np.cos(np.pi * t)
    p = np.arange(1, 128)
    CF1[:, 1:] = -np.sin(ang * np.outer(t, p))
    CF0 = CF0[:, PERM]
    CF1 = CF1[:, PERM]
    cfs = np.stack([
        np.concatenate([CF0[:128], CF0[128:]], axis=1),
        np.concatenate([CF1[:128], CF1[128:]], axis=1),
    ], axis=1).astype(f16)                                  # [128, 2, 256]

    # wpk[(f*16+j), g, c, (f*16+i)] = M_c[i, j, 8g+f];  M = (Wr, Wi, -Wi)
    wpk = np.zeros((128, NG, 3, 128), dtype=f16)
    jj = np.arange(KIN)[:, None, None]
    ii = np.arange(KOUT)[None, :, None]
    ff = np.arange(8)[None, None, :]
    for g in range(NG):
        for c, M in enumerate((W_real, W_imag, -W_imag)):
            wpk[ff * 16 + jj, g, c, ff * 16 + ii] = M[ii, jj, 8 * g + ff]
    wnyq = np.ascontiguousarray(W_real[:, :, 128].T).astype(f16)  # [j, i]

    m = np.arange(BS)
    D0 = np.empty((128, BS))
    D0[0] = 1.0 / BS
    nn = np.arange(1, 128)
    D0[1:] = (2.0 / BS) * np.cos(ang * np.outer(nn, m))
    D1 = np.empty((128, BS))
    D1[0] = ((-1.0) ** m) / BS
    D1[1:] = -(2.0 / BS) * np.sin(ang * np.outer(nn, m))
    dmat = np.stack([D0[PERM], D1[PERM]], axis=1).astype(f16)  # [128, 2, 256]

    ident = np.eye(128, dtype=np.float32)
    return {"cfs": cfs, "wpk": wpk, "wnyq": wnyq, "dmat": dmat, "ident": ident}


def _build_program():
    nc = bacc.Bacc(
        "TRN2", target_bir_lowering=False, debug=False, num_devices=NCORES
    )
    x_d = nc.dram_tensor("x", [BC, D_IN], F32, kind="ExternalInput").ap()
    cfs_d = nc.dram_tensor("cfs", [128, 2, 256], F16, kind="ExternalInput").ap()
    wpk_d = nc.dram_tensor("wpk", [128, NG, 3, 128], F16, kind="ExternalInput").ap()
    wnyq_d = nc.dram_tensor("wnyq", [KIN, KOUT], F16, kind="ExternalInput").ap()
    dmat_d = nc.dram_tensor("dmat", [128, 2, 256], F16, kind="ExternalInput").ap()
    ident_d = nc.dram_tensor("ident", [128, 128], F32, kind="ExternalInput").ap()
    out_d = nc.dram_tensor("out", [BC, D_OUT], F32, kind="ExternalOutput").ap()

    cp_state = [0]

    with tile.TileContext(nc) as tc, ExitStack() as ctx:
        def copy(dst, src):
            # alternate PSUM->SBUF copies between DVE and ACT
            if cp_state[0] % 2 == 0:
                nc.vector.tensor_copy(dst, src)
            else:
                nc.scalar.copy(dst, src)
            cp_state[0] += 1

        consts = ctx.enter_context(tc.tile_pool(name="consts", bufs=1))
        xsp = ctx.enter_context(tc.tile_pool(name="xsp", bufs=2))
        xtp = ctx.enter_context(tc.tile_pool(name="xtp", bufs=1))
        mid1 = ctx.enter_context(tc.tile_pool(name="mid1", bufs=1))
        mid2 = ctx.enter_context(tc.tile_pool(name="mid2", bufs=2))
        osp = ctx.enter_context(tc.tile_pool(name="osp", bufs=2))
        ps = ctx.enter_context(tc.tile_pool(name="ps", bufs=4, space="PSUM"))

        cfs = consts.tile([128, 2, 256], F16)
        wpk = consts.tile([128, NG, 3, 128], F16)
        wnyq = consts.tile([KIN, KOUT], F16)
        dmat = consts.tile([128, 2, 256], F16)
        ident = consts.tile([128, 128], F32)

        # consts ride the scalar (HWDGE-ACT) ring; x loads own sync ring
        nc.scalar.dma_start(cfs[:], cfs_d)
        nc.scalar.dma_start(wpk[:], wpk_d)
        nc.scalar.dma_start(wnyq[:], wnyq_d)
        nc.scalar.dma_start(dmat[:], dmat_d)
        nc.scalar.dma_start(ident[:], ident_d)

        # ---- input loads: 4 chunks of 128 rows on the sync ring
        xs = [xsp.tile([128, D_IN], F32, tag="xs", name=f"xs{i}")
              for i in range(4)]
        for bc in range(4):
            nc.sync.dma_start(xs[bc][:], x_d[128 * bc:128 * (bc + 1), :])

        # per-slab tiles; gg/yh double-buffered so regroup DMAs of slab 1
        # don't wait on slab 0's consumers
        xt = [xtp.tile([128, 32, SLAB], F16, tag="xt", name=f"xt{i}")
              for i in range(2)]
        xf = [mid1.tile([128, KIN, 2, SLAB], F16, tag="xf", name=f"xf{i}")
              for i in range(2)]
        gg = [mid2.tile([128, 2, NG, SLAB], F16, tag="gg", name=f"gg{i}")
              for i in range(2)]
        yy = [mid1.tile([128, NG, 2, SLAB], F16, tag="yy", name=f"yy{i}")
              for i in range(2)]
        yh = [mid2.tile([128, 2, KOUT, SLAB], F16, tag="yh", name=f"yh{i}")
              for i in range(2)]
        gnyq = [consts.tile([KIN, SLAB], F16, tag="gnyq", name=f"gnyq{i}")
                for i in range(2)]

        def stage_T(s, c):
            # transpose chunk bc=2s+c of x into xt[s][:, :, 128c:128c+128]
            bc = 2 * s + c
            for dcg in range(8):
                pt = ps.tile([128, 4, 128], F32, tag="ps")
                for q in range(4):
                    dc = dcg * 4 + q
                    nc.tensor.transpose(
                        pt[:, q, :], xs[bc][:, 128 * dc:128 * (dc + 1)],
                        ident[:],
                    )
                copy(xt[s][:, 4 * dcg:4 * dcg + 4, 128 * c:128 * (c + 1)],
                     pt[:])

        def stage_F(s):
            # real DFT: xf[s][n, j, which, b] = sum_t cfs[t, which, n] xt[t, (j,tc), b]
            # j pairs map to the two banks of one PSUM slot (slots 0 / 2) so
            # accumulation groups never interleave within a bank, while each
            # LDWEIGHTS (cfs half) feeds two matmuls.
            for which in range(2):
                for jp in range(8):
                    pf = ps.tile([128, 4, 256], F32, tag="ps")
                    for tc_ in range(2):
                        for bank in range(2):
                            j = 2 * jp + bank
                            nc.tensor.matmul(
                                pf[:, 2 * bank, :],
                                cfs[:, which, 128 * tc_:128 * (tc_ + 1)],
                                xt[s][:, 2 * j + tc_, :],
                                start=(tc_ == 0),
                                stop=(tc_ == 1),
                            )
                    copy(xf[s][:, 2 * jp:2 * jp + 2, which, :], pf[:, 0::2, :])

        def regroup1(s, eng):
            # gg[s][(f,j), ri, g, b] = xf[s][(f,g), j, ri, b]; one DMA per g
            for g in range(NG):
                eng.dma_start(out=gg[s][:, :, g, :], in_=xf[s][g::16, :, :, :])

        def stage_E(s):
            # per-frequency-group complex mixing
            for g in range(NG):
                pe = ps.tile([128, 4, 256], F32, tag="ps")
                # slot 0 (bank0) = Yr, slot 2 (bank1) = Yi
                nc.tensor.matmul(pe[:, 0, :], wpk[:, g, 0, :],
                                 gg[s][:, 0, g, :], start=True, stop=False)
                nc.tensor.matmul(pe[:, 2, :], wpk[:, g, 0, :],
                                 gg[s][:, 1, g, :], start=True, stop=False)
                nc.tensor.matmul(pe[:, 0, :], wpk[:, g, 1, :],
                                 gg[s][:, 1, g, :], start=False, stop=True)
                nc.tensor.matmul(pe[:, 2, :], wpk[:, g, 2, :],
                                 gg[s][:, 0, g, :], start=False, stop=True)
                copy(yy[s][:, g, :, :], pe[:, 0::2, :])
            # Nyquist einsum lands in the (f=0,g=0) rows of the imag half
            # (the otherwise meaningless Zi[0] slots); regroup2 then routes
            # it to yh[.,1,...] row 0, where dmat row 0 of D1 applies it.
            pyn = ps.tile([KIN, 256], F32, tag="ps")
            nc.tensor.matmul(pyn[:], wnyq[:], gnyq[s][:],
                             start=True, stop=True)
            copy(yy[s][0:KIN, 0, 1, :], pyn[:])

        def regroup2(s, eng):
            # yh[s][(f,g), ri, i, b] = yy[s][(f,i), g, ri, b]; one DMA per i
            for i in range(KOUT):
                eng.dma_start(out=yh[s][:, :, i, :], in_=yy[s][i::16, :, :, :])

        def stage_I(s):
            # inverse DFT with the data stationary -> [b, m] orientation
            for ig in range(4):
                osb = osp.tile([128, 2, 1024], F32, tag="os")
                for iq in range(4):
                    i = 4 * ig + iq
                    po = ps.tile([128, 4, 256], F32, tag="ps")
                    for c in range(2):   # bs chunk -> banks 0 / 1
                        nc.tensor.matmul(
                            po[:, 2 * c, :],
                            yh[s][:, 0, i, 128 * c:128 * (c + 1)],
                            dmat[:, 0, :], start=True, stop=False)
                    for c in range(2):
                        nc.tensor.matmul(
                            po[:, 2 * c, :],
                            yh[s][:, 1, i, 128 * c:128 * (c + 1)],
                            dmat[:, 1, :], start=False, stop=True)
                    copy(osb[:, :, 256 * iq:256 * (iq + 1)], po[:, 0::2, :])
                # store [256 rows, 1024 cols] of out
                eng = nc.sync if s == 0 else nc.scalar
                eng.dma_start(
                    out_d[SLAB * s:SLAB * (s + 1),
                          1024 * ig:1024 * (ig + 1)].rearrange(
                              "(c p) m -> p c m", c=2),
                    osb[:],
                )

        # ---- pipelined emission (per-engine queues in execution order)
        stage_T(0, 0)
        stage_T(0, 1)
        stage_F(0)
        regroup1(0, nc.scalar)
        nc.gpsimd.dma_start(out=gnyq[0][:], in_=xf[0][0:1, :, 1, :])
        stage_T(1, 0)
        stage_T(1, 1)
        stage_F(1)
        regroup1(1, nc.scalar)
        nc.gpsimd.dma_start(out=gnyq[1][:], in_=xf[1][0:1, :, 1, :])
        stage_E(0)
        regroup2(0, nc.sync)
        stage_E(1)
        regroup2(1, nc.scalar)
        stage_I(0)
        stage_I(1)

    nc.compile()
    return nc


def _get_program():
    if "nc" not in _CACHE:
        _CACHE["nc"] = _build_program()
    return _CACHE["nc"]


def _install_ntff_hook():
    """Provide antenv.axon_hooks (absent in this image) so that
    run_bass_kernel_spmd(trace=True) can capture NTFF profiles through the
    axon client library."""
    import sys
    import types
    import ctypes
    import contextlib

    if "antenv.axon_hooks" in sys.modules:
        return
    try:
        lib = ctypes.CDLL("/opt/axon/libaxon_pjrt.so")
    except OSError:
        return
    if not hasattr(lib, "axon_start_nrt_profile"):
        return
    lib.axon_start_nrt_profile.argtypes = [
        ctypes.POINTER(ctypes.c_int64),
        ctypes.c_size_t,
    ]
    lib.axon_start_nrt_profile.restype = ctypes.c_int64
    lib.axon_stop_nrt_profile.argtypes = [ctypes.c_char_p]
    lib.axon_stop_nrt_profile.restype = ctypes.c_int64

    @contextlib.contextmanager
    def _hook(output_dir, device_ids):
        import jax

        jax.devices()
        if device_ids:
            ids = (ctypes.c_int64 * len(device_ids))(*device_ids)
            rc = lib.axon_start_nrt_profile(ids, len(device_ids))
        else:
            rc = lib.axon_start_nrt_profile(None, 0)
        if rc != 0:
            raise RuntimeError(f"axon_start_nrt_profile rc={rc}")
        try:
            yield
        finally:
            n = lib.axon_stop_nrt_profile(str(output_dir).encode())
            print(f"ntff profile: {n} file(s) -> {output_dir}")

    mod = types.ModuleType("antenv.axon_hooks")
    state = {"hook": _hook}
    mod.get_axon_ntff_profile_hook = lambda: state["hook"]
    mod.set_axon_ntff_profile_hook = lambda h: state.update(hook=h)
    sys.modules["antenv.axon_hooks"] = mod
    import antenv

    antenv.axon_hooks = mod


def kernel(x, W_real, W_imag, block_size, out_features):
    global LAST_RESULTS
    x = np.ascontiguousarray(np.asarray(x, dtype=np.float32))
    Wr = np.asarray(W_real, dtype=np.float32)
    Wi = np.asarray(W_imag, dtype=np.float32)
    assert int(block_size) == BS and int(out_features) == D_OUT
    assert x.shape == (B_FULL, D_IN) and Wr.shape == (KOUT, KIN, 129)

    nc = _get_program()
    consts = _build_consts(Wr, Wi)
    core_ids = list(range(NCORES))
    in_maps = [
        {"x": np.ascontiguousarray(x[c * BC:(c + 1) * BC]), **consts}
        for c in core_ids
    ]
    trace = bool(int(os.environ.get("KERNEL_TRACE", "0")))
    if trace:
        _install_ntff_hook()
    res = run_bass_kernel_spmd(nc, in_maps, core_ids, trace=trace)
    LAST_RESULTS = res
    out = np.concatenate([res.results[c]["out"] for c in core_ids], axis=0)
    return np.ascontiguousarray(out.astype(np.float32))


# revision 14
# speedup vs baseline: 1.1062x; 1.0556x over previous
"""Trainium2 Bass kernel for BlockFFTDirectPrior.

Computes out = irfft(einsum('bjn,ijn->bin', rfft(x_blocks), conj(W)))
reshaped to [B, 4096], for x [4096, 4096] f32, W [16, 16, 129] complex
(block size 256).

Strategy: data-parallel over the batch axis across 8 NeuronCores (512 rows
each). Per core, the 512 rows are processed as two 256-row slabs flowing
through a 4-stage PE pipeline so input DMA, compute, the two partition
regroups, and output stores all overlap:

  T: transpose x tiles (PE transpose vs identity)     -> xt [t, dc, b] fp16
  F: real DFT as fp16 matmuls (contract t)            -> xf [n, j, ri, b]
  E: per-frequency 16x16 complex mixing as 8-frequency
     block-diagonal fp16 matmuls (K = (f,j) = 128)    -> yy [n', g, ri, b]
  I: real inverse DFT, data stationary (fp16 weights,
     FWL), which restores [b, m] orientation for free -> out rows

DFT/IDFT row order is swizzled to r = f*16+g so the two partition
regroups between F/E and E/I become per-g (resp. per-i) affine
SBUF->SBUF DMAs that carry both the real and imag halves in one
transfer. Regroups and stores ride the two fast HWDGE rings (sync +
scalar) ordered to match the pipeline; intermediates are fp16, which
halves regroup bytes and doubles LDWEIGHTS rate (FWL). Accumulation
groups are bank-interleaved in PSUM so one LDWEIGHTS feeds two matmuls.
"""

import os
import numpy as np
from contextlib import ExitStack

import concourse.bass as bass
import concourse.tile as tile
from concourse import bacc, mybir
from concourse.bass_utils import run_bass_kernel_spmd

NCORES = 8
B_FULL, D_IN, D_OUT, BS = 4096, 4096, 4096, 256
BC = B_FULL // NCORES          # 512 batch rows per core
SLAB = 256                     # rows per pipeline slab (2 slabs per core)
KIN = KOUT = 16
NG = 16                        # groups of 8 frequencies covering n=0..127
F16 = mybir.dt.float16
F32 = mybir.dt.float32

_CACHE = {}
LAST_RESULTS = None            # BassKernelResults of the most recent run


# DFT/IDFT row swizzle: row r = f*16+g holds frequency n = 8g+f. This makes
# both partition regroups plain affine DMAs (partition dim outermost).
PERM = np.array([8 * (r % 16) + r // 16 for r in range(128)])


def _build_consts(W_real, W_imag):
    """Constant matrices in the exact SBUF layouts the kernel reads."""
    f16 = np.float16
    t = np.arange(BS)
    n0 = np.arange(128)
    ang = 2.0 * np.pi / BS

    CF0 = np.cos(ang * np.outer(t, n0))
    CF1 = np.empty((BS, 128))
    CF1[:, 0] = np.cos(np.pi * t)
    p = np.arange(1, 128)
    CF1[:, 1:] = -np.sin(ang * np.outer(t, p))
    CF0 = CF0[:, PERM]
    CF1 = CF1[:, PERM]
    cfs = np.stack([
        np.concatenate([CF0[:128], CF0[128:]], axis=1),
        np.concatenate([CF1[:128], CF1[128:]], axis=1),
    ], axis=1).astype(f16)                                  # [128, 2, 256]

    # wpk[(f*16+j), g, c, (f*16+i)] = M_c[i, j, 8g+f];  M = (Wr, Wi, -Wi)
    wpk = np.zeros((128, NG, 3, 128), dtype=f16)
    jj = np.arange(KIN)[:, None, None]
    ii = np.arange(KOUT)[None, :, None]
    ff = np.arange(8)[None, None, :]
    for g in range(NG):
        for c, M in enumerate((W_real, W_imag, -W_imag)):
            wpk[ff * 16 + jj, g, c, ff * 16 + ii] = M[ii, jj, 8 * g + ff]
    wnyq = np.ascontiguousarray(W_real[:, :, 128].T).astype(f16)  # [j, i]

    m = np.arange(BS)
    D0 = np.empty((128, BS))
    D0[0] = 1.0 / BS
    nn = np.arange(1, 128)
    D0[1:] = (2.0 / BS) * np.cos(ang * np.outer(nn, m))
    D1 = np.empty((128, BS))
    D1[0] = ((-1.0) ** m) / BS
    D1[1:] = -(2.0 / BS) * np.sin(ang * np.outer(nn, m))
    dmat = np.stack([D0[PERM], D1[PERM]], axis=1).astype(f16)  # [128, 2, 256]

    ident = np.eye(128, dtype=np.float32)
    return {"cfs": cfs, "wpk": wpk, "wnyq": wnyq, "dmat": dmat, "ident": ident}


def _build_program():
    nc = bacc.Bacc(
        "TRN2", target_bir_lowering=False, debug=False, num_devices=NCORES
    )
    x_d = nc.dram_tensor("x", [BC, D_IN], F32, kind="ExternalInput").ap()
    cfs_d = nc.dram_tensor("cfs", [128, 2, 256], F16, kind="ExternalInput").ap()
    wpk_d = nc.dram_tensor("wpk", [128, NG, 3, 128], F16, kind="ExternalInput").ap()
    wnyq_d = nc.dram_tensor("wnyq", [KIN, KOUT], F16, kind="ExternalInput").ap()
    dmat_d = nc.dram_tensor("dmat", [128, 2, 256], F16, kind="ExternalInput").ap()
    ident_d = nc.dram_tensor("ident", [128, 128], F32, kind="ExternalInput").ap()
    out_d = nc.dram_tensor("out", [BC, D_OUT], F32, kind="ExternalOutput").ap()

    cp_state = [0]

    with tile.TileContext(nc) as tc, ExitStack() as ctx:
        def copy(dst, src):
            # alternate PSUM->SBUF copies between DVE and ACT
            if cp_state[0] % 2 == 0:
                nc.vector.tensor_copy(dst, src)
            else:
                nc.scalar.copy(dst, src)
            cp_state[0] += 1

        consts = ctx.enter_context(tc.tile_pool(name="consts", bufs=1))
        xsp = ctx.enter_context(tc.tile_pool(name="xsp", bufs=3))
        xtp = ctx.enter_context(tc.tile_pool(name="xtp", bufs=1))
        mid1 = ctx.enter_context(tc.tile_pool(name="mid1", bufs=1))
        mid2 = ctx.enter_context(tc.tile_pool(name="mid2", bufs=2))
        osp = ctx.enter_context(tc.tile_pool(name="osp", bufs=2))
        ps = ctx.enter_context(tc.tile_pool(name="ps", bufs=4, space="PSUM"))

        cfs = consts.tile([128, 2, 256], F16)
        wpk = consts.tile([128, NG, 3, 128], F16)
        wnyq = consts.tile([KIN, KOUT], F16)
        dmat = consts.tile([128, 2, 256], F16)
        ident = consts.tile([128, 128], F32)

        # consts ride the scalar (HWDGE-ACT) ring; x loads own sync ring
        nc.scalar.dma_start(cfs[:], cfs_d)
        nc.scalar.dma_start(wpk[:], wpk_d)
        nc.scalar.dma_start(wnyq[:], wnyq_d)
        nc.scalar.dma_start(dmat[:], dmat_d)
        nc.scalar.dma_start(ident[:], ident_d)

        # ---- input loads: 4 chunks of 128 rows on the sync ring
        xs = [xsp.tile([128, D_IN], F32, tag="xs", name=f"xs{i}")
              for i in range(4)]
        for bc in range(4):
            nc.sync.dma_start(xs[bc][:], x_d[128 * bc:128 * (bc + 1), :])

        # per-slab tiles; gg/yh double-buffered so regroup DMAs of slab 1
        # don't wait on slab 0's consumers
        xt = [xtp.tile([128, 32, SLAB], F16, tag="xt", name=f"xt{i}")
              for i in range(2)]
        xf = [mid1.tile([128, KIN, 2, SLAB], F16, tag="xf", name=f"xf{i}")
              for i in range(2)]
        gg = [mid2.tile([128, NG, 2, SLAB], F16, tag="gg", name=f"gg{i}")
              for i in range(2)]
        yy = [mid1.tile([128, NG, 2, SLAB], F16, tag="yy", name=f"yy{i}")
              for i in range(2)]
        yh = [mid2.tile([128, KOUT, 2, SLAB], F16, tag="yh", name=f"yh{i}")
              for i in range(2)]
        gnyq = [consts.tile([KIN, SLAB], F16, tag="gnyq", name=f"gnyq{i}")
                for i in range(2)]

        # every regroup spreads its per-group DMAs across all three
        # descriptor-generation units (sync/scalar HWDGE + gpsimd SWDGE)
        RINGS = [nc.sync, nc.gpsimd, nc.scalar]

        def stage_T(s, c):
            # transpose chunk bc=2s+c of x into xt[s][:, :, 128c:128c+128]
            bc = 2 * s + c
            for dcg in range(4):
                pt = ps.tile([128, 8, 128], F32, tag="ps")
                for q in range(8):
                    dc = dcg * 8 + q
                    nc.tensor.transpose(
                        pt[:, q, :], xs[bc][:, 128 * dc:128 * (dc + 1)],
                        ident[:],
                    )
                copy(xt[s][:, 8 * dcg:8 * dcg + 8, 128 * c:128 * (c + 1)],
                     pt[:])

        def stage_F(s):
            # real DFT: xf[s][n, j, which, b] = sum_t cfs[t, which, n] xt[t, (j,tc), b]
            # j pairs map to the two banks of one PSUM slot (slots 0 / 2) so
            # accumulation groups never interleave within a bank, while each
            # LDWEIGHTS (cfs half) feeds two matmuls.
            # slot map: j0->0(bank0), j1->2(bank1), then j2->1(bank0), j3->3
            # (a bank's second group starts only after its first stopped)
            for which in range(2):
                for jg in range(4):
                    pf = ps.tile([128, 4, 256], F32, tag="ps")
                    for half in range(2):
                        for tc_ in range(2):
                            for bank in range(2):
                                j = 4 * jg + 2 * half + bank
                                nc.tensor.matmul(
                                    pf[:, 2 * bank + half, :],
                                    cfs[:, which, 128 * tc_:128 * (tc_ + 1)],
                                    xt[s][:, 2 * j + tc_, :],
                                    start=(tc_ == 0),
                                    stop=(tc_ == 1),
                                )
                    # slots (0,2,1,3) hold j order (0,1,2,3)
                    copy(xf[s][:, 4 * jg:4 * jg + 4, which, :],
                         pf[:].rearrange("p (a b) n -> p b a n", a=2))

        def regroup1(s, r0):
            # gg[s][(f,j), g, ri, b] = xf[s][(f,g), j, ri, b]; one DMA per g,
            # round-robin over the three DGE rings starting at r0
            for g in range(NG):
                RINGS[(r0 + g) % 3].dma_start(
                    out=gg[s][:, g, :, :], in_=xf[s][g::16, :, :, :])

        def stage_E(s):
            # per-frequency-group complex mixing, two g per PSUM slot:
            # g even -> slots 0 (Yr) / 2 (Yi), g odd -> slots 1 / 3
            for gp in range(8):
                pe = ps.tile([128, 4, 256], F32, tag="ps")
                for half in range(2):
                    g = 2 * gp + half
                    nc.tensor.matmul(pe[:, half, :], wpk[:, g, 0, :],
                                     gg[s][:, g, 0, :], start=True, stop=False)
                    nc.tensor.matmul(pe[:, 2 + half, :], wpk[:, g, 0, :],
                                     gg[s][:, g, 1, :], start=True, stop=False)
                    nc.tensor.matmul(pe[:, half, :], wpk[:, g, 1, :],
                                     gg[s][:, g, 1, :], start=False, stop=True)
                    nc.tensor.matmul(pe[:, 2 + half, :], wpk[:, g, 2, :],
                                     gg[s][:, g, 0, :], start=False, stop=True)
                # slots (0,2,1,3) hold (g0 Yr, g0 Yi, g1 Yr, g1 Yi)
                copy(yy[s][:, 2 * gp:2 * gp + 2, :, :],
                     pe[:].rearrange("p (a b) n -> p b a n", a=2))
            # Nyquist einsum lands in the (f=0,g=0) rows of the imag half
            # (the otherwise meaningless Zi[0] slots); regroup2 then routes
            # it to yh[.,1,...] row 0, where dmat row 0 of D1 applies it.
            pyn = ps.tile([KIN, 256], F32, tag="ps")
            nc.tensor.matmul(pyn[:], wnyq[:], gnyq[s][:],
                             start=True, stop=True)
            copy(yy[s][0:KIN, 0, 1, :], pyn[:])

        def regroup2(s, r0):
            # yh[s][(f,g), i, ri, b] = yy[s][(f,i), g, ri, b]; one DMA per i
            for i in range(KOUT):
                RINGS[(r0 + i) % 3].dma_start(
                    out=yh[s][:, i, :, :], in_=yy[s][i::16, :, :, :])

        def stage_I(s):
            # inverse DFT with the data stationary -> [b, m] orientation;
            # two i per PSUM slot: i even -> slots 0 (c=0) / 2 (c=1),
            # i odd -> slots 1 / 3
            for ig in range(4):
                osb = osp.tile([128, 2, 1024], F32, tag="os")
                for ip in range(2):
                    po = ps.tile([128, 4, 256], F32, tag="ps")
                    for half in range(2):
                        i = 4 * ig + 2 * ip + half
                        for c in range(2):   # bs chunk -> banks 0 / 1
                            nc.tensor.matmul(
                                po[:, 2 * c + half, :],
                                yh[s][:, i, 0, 128 * c:128 * (c + 1)],
                                dmat[:, 0, :], start=True, stop=False)
                        for c in range(2):
                            nc.tensor.matmul(
                                po[:, 2 * c + half, :],
                                yh[s][:, i, 1, 128 * c:128 * (c + 1)],
                                dmat[:, 1, :], start=False, stop=True)
                    # slots (0,1,2,3) = (i0c0, i1c0, i0c1, i1c1) = dst order
                    copy(osb[:, :, 512 * ip:512 * (ip + 1)], po[:])
                # store [256 rows, 1024 cols] of out
                eng = nc.sync if s == 0 else nc.scalar
                eng.dma_start(
                    out_d[SLAB * s:SLAB * (s + 1),
                          1024 * ig:1024 * (ig + 1)].rearrange(
                              "(c p) m -> p c m", c=2),
                    osb[:],
                )

        # ---- pipelined emission (per-engine queues in execution order)
        stage_T(0, 0)
        stage_T(0, 1)
        stage_F(0)
        regroup1(0, 1)
        nc.gpsimd.dma_start(out=gnyq[0][:], in_=xf[0][0:1, :, 1, :])
        stage_T(1, 0)
        stage_T(1, 1)
        stage_F(1)
        regroup1(1, 0)
        nc.gpsimd.dma_start(out=gnyq[1][:], in_=xf[1][0:1, :, 1, :])
        stage_E(0)
        regroup2(0, 2)
        stage_E(1)
        regroup2(1, 0)
        stage_I(0)
        stage_I(1)

    nc.compile()
    return nc


def _get_program():
    if "nc" not in _CACHE:
        _CACHE["nc"] = _build_program()
    return _CACHE["nc"]


def _install_ntff_hook():
    """Provide antenv.axon_hooks (absent in this image) so that
    run_bass_kernel_spmd(trace=True) can capture NTFF profiles through the
    axon client library."""
    import sys
    import types
    import ctypes
    import contextlib

    if "antenv.axon_hooks" in sys.modules:
        return
    try:
        lib = ctypes.CDLL("/opt/axon/libaxon_pjrt.so")
    except OSError:
        return
    if not hasattr(lib, "axon_start_nrt_profile"):
        return
    lib.axon_start_nrt_profile.argtypes = [
        ctypes.POINTER(ctypes.c_int64),
        ctypes.c_size_t,
    ]
    lib.axon_start_nrt_profile.restype = ctypes.c_int64
    lib.axon_stop_nrt_profile.argtypes = [ctypes.c_char_p]
    lib.axon_stop_nrt_profile.restype = ctypes.c_int64

    @contextlib.contextmanager
    def _hook(output_dir, device_ids):
        import jax

        jax.devices()
        if device_ids:
            ids = (ctypes.c_int64 * len(device_ids))(*device_ids)
            rc = lib.axon_start_nrt_profile(ids, len(device_ids))
        else:
            rc = lib.axon_start_nrt_profile(None, 0)
        if rc != 0:
            raise RuntimeError(f"axon_start_nrt_profile rc={rc}")
        try:
            yield
        finally:
            n = lib.axon_stop_nrt_profile(str(output_dir).encode())
            print(f"ntff profile: {n} file(s) -> {output_dir}")

    mod = types.ModuleType("antenv.axon_hooks")
    state = {"hook": _hook}
    mod.get_axon_ntff_profile_hook = lambda: state["hook"]
    mod.set_axon_ntff_profile_hook = lambda h: state.update(hook=h)
    sys.modules["antenv.axon_hooks"] = mod
    import antenv

    antenv.axon_hooks = mod


def kernel(x, W_real, W_imag, block_size, out_features):
    global LAST_RESULTS
    x = np.ascontiguousarray(np.asarray(x, dtype=np.float32))
    Wr = np.asarray(W_real, dtype=np.float32)
    Wi = np.asarray(W_imag, dtype=np.float32)
    assert int(block_size) == BS and int(out_features) == D_OUT
    assert x.shape == (B_FULL, D_IN) and Wr.shape == (KOUT, KIN, 129)

    nc = _get_program()
    consts = _build_consts(Wr, Wi)
    core_ids = list(range(NCORES))
    in_maps = [
        {"x": np.ascontiguousarray(x[c * BC:(c + 1) * BC]), **consts}
        for c in core_ids
    ]
    trace = bool(int(os.environ.get("KERNEL_TRACE", "0")))
    if trace:
        _install_ntff_hook()
    res = run_bass_kernel_spmd(nc, in_maps, core_ids, trace=trace)
    LAST_RESULTS = res
    out = np.concatenate([res.results[c]["out"] for c in core_ids], axis=0)
    return np.ascontiguousarray(out.astype(np.float32))


# revision 17
# speedup vs baseline: 1.2462x; 1.1265x over previous
"""Trainium2 Bass kernel for BlockFFTDirectPrior.

Computes out = irfft(einsum('bjn,ijn->bin', rfft(x_blocks), conj(W)))
reshaped to [B, 4096], for x [4096, 4096] f32, W [16, 16, 129] complex
(block size 256).

Strategy: data-parallel over the batch axis across 8 NeuronCores (512 rows
each). Per core, the 512 rows are processed as two 256-row slabs flowing
through a 4-stage PE pipeline so input DMA, compute, the two partition
regroups, and output stores all overlap:

  T: transpose x tiles (PE transpose vs identity)     -> xt [t, dc, b] fp16
  F: real DFT as fp16 matmuls (contract t)            -> xf [n, j, ri, b]
  E: per-frequency 16x16 complex mixing as 8-frequency
     block-diagonal fp16 matmuls (K = (f,j) = 128)    -> yy [n', g, ri, b]
  I: real inverse DFT, data stationary (fp16 weights,
     FWL), which restores [b, m] orientation for free -> out rows

DFT/IDFT row order is swizzled to r = f*16+g so the two partition
regroups between F/E and E/I become per-g (resp. per-i) affine
SBUF->SBUF DMAs that carry both the real and imag halves in one
transfer. Regroups and stores ride the two fast HWDGE rings (sync +
scalar) ordered to match the pipeline; intermediates are fp16, which
halves regroup bytes and doubles LDWEIGHTS rate (FWL). Accumulation
groups are bank-interleaved in PSUM so one LDWEIGHTS feeds two matmuls.
"""

import os
import numpy as np
from contextlib import ExitStack

import concourse.bass as bass
import concourse.tile as tile
from concourse import bacc, mybir
from concourse.bass_utils import run_bass_kernel_spmd

NCORES = 8
B_FULL, D_IN, D_OUT, BS = 4096, 4096, 4096, 256
BC = B_FULL // NCORES          # 512 batch rows per core
SLAB = 256                     # rows per pipeline slab (2 slabs per core)
KIN = KOUT = 16
NG = 16                        # groups of 8 frequencies covering n=0..127
F16 = mybir.dt.float16
F32 = mybir.dt.float32

_CACHE = {}
LAST_RESULTS = None            # BassKernelResults of the most recent run


# DFT/IDFT row swizzle: row r = f*16+g holds frequency n = 8g+f. This makes
# both partition regroups plain affine DMAs (partition dim outermost).
PERM = np.array([8 * (r % 16) + r // 16 for r in range(128)])


def _build_consts(W_real, W_imag):
    """Constant matrices in the exact SBUF layouts the kernel reads."""
    f16 = np.float16
    t = np.arange(BS)
    n0 = np.arange(128)
    ang = 2.0 * np.pi / BS

    CF0 = np.cos(ang * np.outer(t, n0))
    CF1 = np.empty((BS, 128))
    CF1[:, 0] = np.cos(np.pi * t)
    p = np.arange(1, 128)
    CF1[:, 1:] = -np.sin(ang * np.outer(t, p))
    CF0 = CF0[:, PERM]
    CF1 = CF1[:, PERM]
    cfs = np.stack([
        np.concatenate([CF0[:128], CF0[128:]], axis=1),
        np.concatenate([CF1[:128], CF1[128:]], axis=1),
    ], axis=1).astype(f16)                                  # [128, 2, 256]

    # wpk[(f*16+j), g, c, (f*16+i)] = M_c[i, j, 8g+f];  M = (Wr, Wi, -Wi)
    wpk = np.zeros((128, NG, 3, 128), dtype=f16)
    jj = np.arange(KIN)[:, None, None]
    ii = np.arange(KOUT)[None, :, None]
    ff = np.arange(8)[None, None, :]
    for g in range(NG):
        for c, M in enumerate((W_real, W_imag, -W_imag)):
            wpk[ff * 16 + jj, g, c, ff * 16 + ii] = M[ii, jj, 8 * g + ff]
    wnyq = np.ascontiguousarray(W_real[:, :, 128].T).astype(f16)  # [j, i]

    m = np.arange(BS)
    D0 = np.empty((128, BS))
    D0[0] = 1.0 / BS
    nn = np.arange(1, 128)
    D0[1:] = (2.0 / BS) * np.cos(ang * np.outer(nn, m))
    D1 = np.empty((128, BS))
    D1[0] = ((-1.0) ** m) / BS
    D1[1:] = -(2.0 / BS) * np.sin(ang * np.outer(nn, m))
    dmat = np.stack([D0[PERM], D1[PERM]], axis=1).astype(f16)  # [128, 2, 256]

    ident = np.eye(128, dtype=np.float32)
    return {"cfs": cfs, "wpk": wpk, "wnyq": wnyq, "dmat": dmat, "ident": ident}


def _build_program():
    nc = bacc.Bacc(
        "TRN2", target_bir_lowering=False, debug=False, num_devices=NCORES
    )
    x_d = nc.dram_tensor("x", [BC, D_IN], F32, kind="ExternalInput").ap()
    cfs_d = nc.dram_tensor("cfs", [128, 2, 256], F16, kind="ExternalInput").ap()
    wpk_d = nc.dram_tensor("wpk", [128, NG, 3, 128], F16, kind="ExternalInput").ap()
    wnyq_d = nc.dram_tensor("wnyq", [KIN, KOUT], F16, kind="ExternalInput").ap()
    dmat_d = nc.dram_tensor("dmat", [128, 2, 256], F16, kind="ExternalInput").ap()
    ident_d = nc.dram_tensor("ident", [128, 128], F32, kind="ExternalInput").ap()
    out_d = nc.dram_tensor("out", [BC, D_OUT], F32, kind="ExternalOutput").ap()

    cp_state = [0]

    with tile.TileContext(nc) as tc, ExitStack() as ctx:
        def copy(dst, src):
            # alternate PSUM->SBUF copies between DVE and ACT
            if cp_state[0] % 2 == 0:
                nc.vector.tensor_copy(dst, src)
            else:
                nc.scalar.copy(dst, src)
            cp_state[0] += 1

        consts = ctx.enter_context(tc.tile_pool(name="consts", bufs=1))
        xsp = ctx.enter_context(tc.tile_pool(name="xsp", bufs=3))
        xtp = ctx.enter_context(tc.tile_pool(name="xtp", bufs=1))
        mid1 = ctx.enter_context(tc.tile_pool(name="mid1", bufs=1))
        mid2 = ctx.enter_context(tc.tile_pool(name="mid2", bufs=2))
        osp = ctx.enter_context(tc.tile_pool(name="osp", bufs=2))
        ps = ctx.enter_context(tc.tile_pool(name="ps", bufs=4, space="PSUM"))

        cfs = consts.tile([128, 2, 256], F16)
        wpk = consts.tile([128, NG, 3, 128], F16)
        wnyq = consts.tile([KIN, KOUT], F16)
        dmat = consts.tile([128, 2, 256], F16)
        ident = consts.tile([128, 128], F32)

        # ident (needed by the first transposes) loads first on the fast
        # sync ring; bulky-but-late consts ride the gpsimd (SWDGE) ring
        nc.sync.dma_start(ident[:], ident_d)
        nc.sync.dma_start(cfs[:], cfs_d)
        nc.gpsimd.dma_start(wpk[:], wpk_d)
        nc.gpsimd.dma_start(dmat[:], dmat_d)
        nc.gpsimd.dma_start(wnyq[:], wnyq_d)

        # ---- input loads: 4 chunks of 128 rows x 2 column halves on the
        # sync ring (column split lets transposes start after half a chunk)
        xs = [xsp.tile([128, D_IN], F32, tag="xs", name=f"xs{i}")
              for i in range(4)]
        for bc in range(4):
            for h in range(2):
                nc.sync.dma_start(
                    xs[bc][:, 2048 * h:2048 * (h + 1)],
                    x_d[128 * bc:128 * (bc + 1), 2048 * h:2048 * (h + 1)])

        # per-slab tiles; gg/yh double-buffered so regroup DMAs of slab 1
        # don't wait on slab 0's consumers
        xt = [xtp.tile([128, 32, SLAB], F16, tag="xt", name=f"xt{i}")
              for i in range(2)]
        xf = [mid1.tile([128, KIN, 2, SLAB], F16, tag="xf", name=f"xf{i}")
              for i in range(2)]
        gg = [mid2.tile([128, NG, 2, SLAB], F16, tag="gg", name=f"gg{i}")
              for i in range(2)]
        yy = [mid1.tile([128, NG, 2, SLAB], F16, tag="yy", name=f"yy{i}")
              for i in range(2)]
        yh = [mid2.tile([128, KOUT, 2, SLAB], F16, tag="yh", name=f"yh{i}")
              for i in range(2)]
        gnyq = [consts.tile([KIN, SLAB], F16, tag="gnyq", name=f"gnyq{i}")
                for i in range(2)]

        # every regroup spreads its per-group DMAs across the three
        # descriptor-generation units, weighted toward the faster rings
        # (sync HWDGE 7 : gpsimd SWDGE 6 : scalar HWDGE 3)
        RR = ([nc.sync, nc.gpsimd] * 6 + [nc.sync, nc.scalar] +
              [nc.scalar, nc.sync])

        def stage_T(s, c):
            # transpose chunk bc=2s+c of x into xt[s][:, :, 128c:128c+128]
            bc = 2 * s + c
            for dcg in range(4):
                pt = ps.tile([128, 8, 128], F32, tag="ps")
                for q in range(8):
                    dc = dcg * 8 + q
                    nc.tensor.transpose(
                        pt[:, q, :], xs[bc][:, 128 * dc:128 * (dc + 1)],
                        ident[:],
                    )
                copy(xt[s][:, 8 * dcg:8 * dcg + 8, 128 * c:128 * (c + 1)],
                     pt[:])

        def stage_F(s):
            # real DFT: xf[s][n, j, which, b] = sum_t cfs[t, which, n] xt[t, (j,tc), b]
            # j pairs map to the two banks of one PSUM slot (slots 0 / 2) so
            # accumulation groups never interleave within a bank, while each
            # LDWEIGHTS (cfs half) feeds two matmuls.
            # slot map: j0->0(bank0), j1->2(bank1), then j2->1(bank0), j3->3
            # (a bank's second group starts only after its first stopped)
            for which in range(2):
                for jg in range(4):
                    pf = ps.tile([128, 4, 256], F32, tag="ps")
                    for half in range(2):
                        for tc_ in range(2):
                            for bank in range(2):
                                j = 4 * jg + 2 * half + bank
                                nc.tensor.matmul(
                                    pf[:, 2 * bank + half, :],
                                    cfs[:, which, 128 * tc_:128 * (tc_ + 1)],
                                    xt[s][:, 2 * j + tc_, :],
                                    start=(tc_ == 0),
                                    stop=(tc_ == 1),
                                )
                    # slots (0,2,1,3) hold j order (0,1,2,3)
                    copy(xf[s][:, 4 * jg:4 * jg + 4, which, :],
                         pf[:].rearrange("p (a b) n -> p b a n", a=2))

        def regroup1(s, r0):
            # gg[s][(f,j), g, ri, b] = xf[s][(f,g), j, ri, b]; one DMA per g,
            # spread over the three DGE rings starting at offset r0
            for g in range(NG):
                RR[(r0 + g) % 16].dma_start(
                    out=gg[s][:, g, :, :], in_=xf[s][g::16, :, :, :])

        def stage_E(s):
            # per-frequency-group complex mixing, two g per PSUM slot:
            # g even -> slots 0 (Yr) / 2 (Yi), g odd -> slots 1 / 3
            for gp in range(8):
                pe = ps.tile([128, 4, 256], F32, tag="ps")
                for half in range(2):
                    g = 2 * gp + half
                    nc.tensor.matmul(pe[:, half, :], wpk[:, g, 0, :],
                                     gg[s][:, g, 0, :], start=True, stop=False)
                    nc.tensor.matmul(pe[:, 2 + half, :], wpk[:, g, 0, :],
                                     gg[s][:, g, 1, :], start=True, stop=False)
                    nc.tensor.matmul(pe[:, half, :], wpk[:, g, 1, :],
                                     gg[s][:, g, 1, :], start=False, stop=True)
                    nc.tensor.matmul(pe[:, 2 + half, :], wpk[:, g, 2, :],
                                     gg[s][:, g, 0, :], start=False, stop=True)
                # slots (0,2,1,3) hold (g0 Yr, g0 Yi, g1 Yr, g1 Yi)
                copy(yy[s][:, 2 * gp:2 * gp + 2, :, :],
                     pe[:].rearrange("p (a b) n -> p b a n", a=2))
            # Nyquist einsum lands in the (f=0,g=0) rows of the imag half
            # (the otherwise meaningless Zi[0] slots); regroup2 then routes
            # it to yh[.,1,...] row 0, where dmat row 0 of D1 applies it.
            pyn = ps.tile([KIN, 256], F32, tag="ps")
            nc.tensor.matmul(pyn[:], wnyq[:], gnyq[s][:],
                             start=True, stop=True)
            copy(yy[s][0:KIN, 0, 1, :], pyn[:])

        def regroup2(s, r0):
            # yh[s][(f,g), i, ri, b] = yy[s][(f,i), g, ri, b]; one DMA per i
            for i in range(KOUT):
                RR[(r0 + i) % 16].dma_start(
                    out=yh[s][:, i, :, :], in_=yy[s][i::16, :, :, :])

        def stage_I(s):
            # inverse DFT with the data stationary -> [b, m] orientation;
            # two i per PSUM slot: i even -> slots 0 (c=0) / 2 (c=1),
            # i odd -> slots 1 / 3
            for ig in range(4):
                osb = osp.tile([128, 2, 1024], F32, tag="os")
                for ip in range(2):
                    po = ps.tile([128, 4, 256], F32, tag="ps")
                    for half in range(2):
                        i = 4 * ig + 2 * ip + half
                        for c in range(2):   # bs chunk -> banks 0 / 1
                            nc.tensor.matmul(
                                po[:, 2 * c + half, :],
                                yh[s][:, i, 0, 128 * c:128 * (c + 1)],
                                dmat[:, 0, :], start=True, stop=False)
                        for c in range(2):
                            nc.tensor.matmul(
                                po[:, 2 * c + half, :],
                                yh[s][:, i, 1, 128 * c:128 * (c + 1)],
                                dmat[:, 1, :], start=False, stop=True)
                    # slots (0,1,2,3) = (i0c0, i1c0, i0c1, i1c1) = dst order
                    copy(osb[:, :, 512 * ip:512 * (ip + 1)], po[:])
                # store [256 rows, 1024 cols] of out
                eng = nc.sync if s == 0 else nc.scalar
                eng.dma_start(
                    out_d[SLAB * s:SLAB * (s + 1),
                          1024 * ig:1024 * (ig + 1)].rearrange(
                              "(c p) m -> p c m", c=2),
                    osb[:],
                )

        # ---- pipelined emission (per-engine queues in execution order)
        stage_T(0, 0)
        stage_T(0, 1)
        stage_F(0)
        regroup1(0, 1)
        nc.gpsimd.dma_start(out=gnyq[0][:], in_=xf[0][0:1, :, 1, :])
        stage_T(1, 0)
        stage_T(1, 1)
        stage_F(1)
        regroup1(1, 0)
        nc.gpsimd.dma_start(out=gnyq[1][:], in_=xf[1][0:1, :, 1, :])
        stage_E(0)
        regroup2(0, 2)
        stage_E(1)
        regroup2(1, 0)
        stage_I(0)
        stage_I(1)

    nc.compile()
    return nc


def _get_program():
    if "nc" not in _CACHE:
        _CACHE["nc"] = _build_program()
    return _CACHE["nc"]


def _install_ntff_hook():
    """Provide antenv.axon_hooks (absent in this image) so that
    run_bass_kernel_spmd(trace=True) can capture NTFF profiles through the
    axon client library."""
    import sys
    import types
    import ctypes
    import contextlib

    if "antenv.axon_hooks" in sys.modules:
        return
    try:
        lib = ctypes.CDLL("/opt/axon/libaxon_pjrt.so")
    except OSError:
        return
    if not hasattr(lib, "axon_start_nrt_profile"):
        return
    lib.axon_start_nrt_profile.argtypes = [
        ctypes.POINTER(ctypes.c_int64),
        ctypes.c_size_t,
    ]
    lib.axon_start_nrt_profile.restype = ctypes.c_int64
    lib.axon_stop_nrt_profile.argtypes = [ctypes.c_char_p]
    lib.axon_stop_nrt_profile.restype = ctypes.c_int64

    @contextlib.contextmanager
    def _hook(output_dir, device_ids):
        import jax

        jax.devices()
        if device_ids:
            ids = (ctypes.c_int64 * len(device_ids))(*device_ids)
            rc = lib.axon_start_nrt_profile(ids, len(device_ids))
        else:
            rc = lib.axon_start_nrt_profile(None, 0)
        if rc != 0:
            raise RuntimeError(f"axon_start_nrt_profile rc={rc}")
        try:
            yield
        finally:
            n = lib.axon_stop_nrt_profile(str(output_dir).encode())
            print(f"ntff profile: {n} file(s) -> {output_dir}")

    mod = types.ModuleType("antenv.axon_hooks")
    state = {"hook": _hook}
    mod.get_axon_ntff_profile_hook = lambda: state["hook"]
    mod.set_axon_ntff_profile_hook = lambda h: state.update(hook=h)
    sys.modules["antenv.axon_hooks"] = mod
    import antenv

    antenv.axon_hooks = mod


def kernel(x, W_real, W_imag, block_size, out_features):
    global LAST_RESULTS
    x = np.ascontiguousarray(np.asarray(x, dtype=np.float32))
    Wr = np.asarray(W_real, dtype=np.float32)
    Wi = np.asarray(W_imag, dtype=np.float32)
    assert int(block_size) == BS and int(out_features) == D_OUT
    assert x.shape == (B_FULL, D_IN) and Wr.shape == (KOUT, KIN, 129)

    nc = _get_program()
    consts = _build_consts(Wr, Wi)
    core_ids = list(range(NCORES))
    in_maps = [
        {"x": np.ascontiguousarray(x[c * BC:(c + 1) * BC]), **consts}
        for c in core_ids
    ]
    trace = bool(int(os.environ.get("KERNEL_TRACE", "0")))
    if trace:
        _install_ntff_hook()
    res = run_bass_kernel_spmd(nc, in_maps, core_ids, trace=trace)
    LAST_RESULTS = res
    out = np.concatenate([res.results[c]["out"] for c in core_ids], axis=0)
    return np.ascontiguousarray(out.astype(np.float32))


# revision 20
# speedup vs baseline: 1.2650x; 1.0151x over previous
"""Trainium2 Bass kernel for BlockFFTDirectPrior.

Computes out = irfft(einsum('bjn,ijn->bin', rfft(x_blocks), conj(W)))
reshaped to [B, 4096], for x [4096, 4096] f32, W [16, 16, 129] complex
(block size 256).

Strategy: data-parallel over the batch axis across 8 NeuronCores (512 rows
each). Per core, the 512 rows are processed as two 256-row slabs flowing
through a 4-stage PE pipeline so input DMA, compute, the two partition
regroups, and output stores all overlap:

  T: transpose x tiles (PE transpose vs identity)     -> xt [t, dc, b] fp16
  F: real DFT as fp16 matmuls (contract t)            -> xf [n, j, ri, b]
  E: per-frequency 16x16 complex mixing as 8-frequency
     block-diagonal fp16 matmuls (K = (f,j) = 128)    -> yy [n', g, ri, b]
  I: real inverse DFT, data stationary (fp16 weights,
     FWL), which restores [b, m] orientation for free -> out rows

DFT/IDFT row order is swizzled to r = f*16+g so the two partition
regroups between F/E and E/I become per-g (resp. per-i) affine
SBUF->SBUF DMAs that carry both the real and imag halves in one
transfer. Regroups and stores ride the two fast HWDGE rings (sync +
scalar) ordered to match the pipeline; intermediates are fp16, which
halves regroup bytes and doubles LDWEIGHTS rate (FWL). Accumulation
groups are bank-interleaved in PSUM so one LDWEIGHTS feeds two matmuls.
"""

import os
import numpy as np
from contextlib import ExitStack

import concourse.bass as bass
import concourse.tile as tile
from concourse import bacc, mybir
from concourse.bass_utils import run_bass_kernel_spmd

NCORES = 8
B_FULL, D_IN, D_OUT, BS = 4096, 4096, 4096, 256
BC = B_FULL // NCORES          # 512 batch rows per core
SLAB = 256                     # rows per pipeline slab (2 slabs per core)
KIN = KOUT = 16
NG = 16                        # groups of 8 frequencies covering n=0..127
F16 = mybir.dt.float16
F32 = mybir.dt.float32

_CACHE = {}
LAST_RESULTS = None            # BassKernelResults of the most recent run


# DFT/IDFT row swizzle: row r = f*16+g holds frequency n = 8g+f. This makes
# both partition regroups plain affine DMAs (partition dim outermost).
PERM = np.array([8 * (r % 16) + r // 16 for r in range(128)])


def _build_consts(W_real, W_imag):
    """Constant matrices in the exact SBUF layouts the kernel reads."""
    f16 = np.float16
    t = np.arange(BS)
    n0 = np.arange(128)
    ang = 2.0 * np.pi / BS

    CF0 = np.cos(ang * np.outer(t, n0))
    CF1 = np.empty((BS, 128))
    CF1[:, 0] = np.cos(np.pi * t)
    p = np.arange(1, 128)
    CF1[:, 1:] = -np.sin(ang * np.outer(t, p))
    CF0 = CF0[:, PERM]
    CF1 = CF1[:, PERM]
    cfs = np.stack([
        np.concatenate([CF0[:128], CF0[128:]], axis=1),
        np.concatenate([CF1[:128], CF1[128:]], axis=1),
    ], axis=1).astype(f16)                                  # [128, 2, 256]

    # wpk[(f*16+j), g, c, (f*16+i)] = M_c[i, j, 8g+f];  M = (Wr, Wi, -Wi)
    wpk = np.zeros((128, NG, 3, 128), dtype=f16)
    jj = np.arange(KIN)[:, None, None]
    ii = np.arange(KOUT)[None, :, None]
    ff = np.arange(8)[None, None, :]
    for g in range(NG):
        for c, M in enumerate((W_real, W_imag, -W_imag)):
            wpk[ff * 16 + jj, g, c, ff * 16 + ii] = M[ii, jj, 8 * g + ff]
    wnyq = np.ascontiguousarray(W_real[:, :, 128].T).astype(f16)  # [j, i]

    m = np.arange(BS)
    D0 = np.empty((128, BS))
    D0[0] = 1.0 / BS
    nn = np.arange(1, 128)
    D0[1:] = (2.0 / BS) * np.cos(ang * np.outer(nn, m))
    D1 = np.empty((128, BS))
    D1[0] = ((-1.0) ** m) / BS
    D1[1:] = -(2.0 / BS) * np.sin(ang * np.outer(nn, m))
    dmat = np.stack([D0[PERM], D1[PERM]], axis=1).astype(f16)  # [128, 2, 256]

    ident = np.eye(128, dtype=np.float32)
    return {"cfs": cfs, "wpk": wpk, "wnyq": wnyq, "dmat": dmat, "ident": ident}


def _build_program():
    nc = bacc.Bacc(
        "TRN2", target_bir_lowering=False, debug=False, num_devices=NCORES
    )
    x_d = nc.dram_tensor("x", [BC, D_IN], F32, kind="ExternalInput").ap()
    cfs_d = nc.dram_tensor("cfs", [128, 2, 256], F16, kind="ExternalInput").ap()
    wpk_d = nc.dram_tensor("wpk", [128, NG, 3, 128], F16, kind="ExternalInput").ap()
    wnyq_d = nc.dram_tensor("wnyq", [KIN, KOUT], F16, kind="ExternalInput").ap()
    dmat_d = nc.dram_tensor("dmat", [128, 2, 256], F16, kind="ExternalInput").ap()
    ident_d = nc.dram_tensor("ident", [128, 128], F32, kind="ExternalInput").ap()
    out_d = nc.dram_tensor("out", [BC, D_OUT], F32, kind="ExternalOutput").ap()

    cp_state = [0]

    with tile.TileContext(nc) as tc, ExitStack() as ctx:
        def copy(dst, src):
            # alternate PSUM->SBUF copies between DVE and ACT
            if cp_state[0] % 2 == 0:
                nc.vector.tensor_copy(dst, src)
            else:
                nc.scalar.copy(dst, src)
            cp_state[0] += 1

        consts = ctx.enter_context(tc.tile_pool(name="consts", bufs=1))
        xsp = ctx.enter_context(tc.tile_pool(name="xsp", bufs=3))
        xtp = ctx.enter_context(tc.tile_pool(name="xtp", bufs=1))
        mid1 = ctx.enter_context(tc.tile_pool(name="mid1", bufs=1))
        mid2 = ctx.enter_context(tc.tile_pool(name="mid2", bufs=2))
        osp = ctx.enter_context(tc.tile_pool(name="osp", bufs=2))
        ps = ctx.enter_context(tc.tile_pool(name="ps", bufs=4, space="PSUM"))

        cfs = consts.tile([128, 2, 256], F16)
        wpk = consts.tile([128, NG, 3, 128], F16)
        wnyq = consts.tile([KIN, KOUT], F16)
        dmat = consts.tile([128, 2, 256], F16)
        ident = consts.tile([128, 128], F32)

        # ident (needed by the first transposes) loads first on the fast
        # sync ring; bulky-but-late consts ride the gpsimd (SWDGE) ring
        nc.sync.dma_start(ident[:], ident_d)
        nc.sync.dma_start(cfs[:], cfs_d)
        nc.gpsimd.dma_start(wpk[:], wpk_d)
        nc.gpsimd.dma_start(dmat[:], dmat_d)
        nc.gpsimd.dma_start(wnyq[:], wnyq_d)

        # ---- input loads: 4 chunks of 128 rows x 2 column halves; slab 0
        # chunks on sync, slab 1 chunks on scalar so later regroup DMAs on
        # sync never queue behind the second slab's input
        xs = [xsp.tile([128, D_IN], F32, tag="xs", name=f"xs{i}")
              for i in range(4)]
        for bc in range(4):
            eng = nc.sync if bc < 2 else nc.scalar
            for h in range(2):
                eng.dma_start(
                    xs[bc][:, 2048 * h:2048 * (h + 1)],
                    x_d[128 * bc:128 * (bc + 1), 2048 * h:2048 * (h + 1)])

        # per-slab tiles; gg/yh double-buffered so regroup DMAs of slab 1
        # don't wait on slab 0's consumers
        xt = [xtp.tile([128, 32, SLAB], F16, tag="xt", name=f"xt{i}")
              for i in range(2)]
        xf = [mid1.tile([128, KIN, 2, SLAB], F16, tag="xf", name=f"xf{i}")
              for i in range(2)]
        # gg/yh are split into halves so E/I can start once the first half
        # of a regroup has landed instead of waiting for all 16 DMAs
        gg = [[mid2.tile([128, NG // 2, 2, SLAB], F16, tag=f"gg{h}",
                         name=f"gg{i}_{h}") for h in range(2)]
              for i in range(2)]
        yy = [mid1.tile([128, NG, 2, SLAB], F16, tag="yy", name=f"yy{i}")
              for i in range(2)]
        yh = [[mid2.tile([128, KOUT // 2, 2, SLAB], F16, tag=f"yh{h}",
                         name=f"yh{i}_{h}") for h in range(2)]
              for i in range(2)]
        gnyq = [consts.tile([KIN, SLAB], F16, tag="gnyq", name=f"gnyq{i}")
                for i in range(2)]

        # every regroup spreads its per-group DMAs across the three
        # descriptor-generation units; gpsimd (SWDGE) issues fastest and
        # has no other duties, so it takes half of each regroup
        RR = [nc.gpsimd, nc.sync, nc.gpsimd, nc.scalar] * 4

        def stage_T(s, c):
            # transpose chunk bc=2s+c of x into xt[s][:, :, 128c:128c+128]
            bc = 2 * s + c
            for dcg in range(4):
                pt = ps.tile([128, 8, 128], F32, tag="ps")
                for q in range(8):
                    dc = dcg * 8 + q
                    nc.tensor.transpose(
                        pt[:, q, :], xs[bc][:, 128 * dc:128 * (dc + 1)],
                        ident[:],
                    )
                copy(xt[s][:, 8 * dcg:8 * dcg + 8, 128 * c:128 * (c + 1)],
                     pt[:])

        def stage_F(s):
            # real DFT: xf[s][n, j, which, b] = sum_t cfs[t, which, n] xt[t, (j,tc), b]
            # j pairs map to the two banks of one PSUM slot (slots 0 / 2) so
            # accumulation groups never interleave within a bank, while each
            # LDWEIGHTS (cfs half) feeds two matmuls.
            # slot map: j0->0(bank0), j1->2(bank1), then j2->1(bank0), j3->3
            # (a bank's second group starts only after its first stopped)
            for which in range(2):
                for jg in range(4):
                    pf = ps.tile([128, 4, 256], F32, tag="ps")
                    for half in range(2):
                        for tc_ in range(2):
                            for bank in range(2):
                                j = 4 * jg + 2 * half + bank
                                nc.tensor.matmul(
                                    pf[:, 2 * bank + half, :],
                                    cfs[:, which, 128 * tc_:128 * (tc_ + 1)],
                                    xt[s][:, 2 * j + tc_, :],
                                    start=(tc_ == 0),
                                    stop=(tc_ == 1),
                                )
                    # slots (0,2,1,3) hold j order (0,1,2,3)
                    copy(xf[s][:, 4 * jg:4 * jg + 4, which, :],
                         pf[:].rearrange("p (a b) n -> p b a n", a=2))

        def regroup1(s, r0):
            # gg[s][(f,j), g, ri, b] = xf[s][(f,g), j, ri, b]; one DMA per g,
            # spread over the three DGE rings starting at offset r0
            for g in range(NG):
                RR[(r0 + g) % 16].dma_start(
                    out=gg[s][g // 8][:, g % 8, :, :],
                    in_=xf[s][g::16, :, :, :])

        def stage_E(s):
            # per-frequency-group complex mixing, two g per PSUM slot:
            # g even -> slots 0 (Yr) / 2 (Yi), g odd -> slots 1 / 3
            for gp in range(8):
                pe = ps.tile([128, 4, 256], F32, tag="ps")
                for half in range(2):
                    g = 2 * gp + half
                    nc.tensor.matmul(pe[:, half, :], wpk[:, g, 0, :],
                                     gg[s][g // 8][:, g % 8, 0, :], start=True, stop=False)
                    nc.tensor.matmul(pe[:, 2 + half, :], wpk[:, g, 0, :],
                                     gg[s][g // 8][:, g % 8, 1, :], start=True, stop=False)
                    nc.tensor.matmul(pe[:, half, :], wpk[:, g, 1, :],
                                     gg[s][g // 8][:, g % 8, 1, :], start=False, stop=True)
                    nc.tensor.matmul(pe[:, 2 + half, :], wpk[:, g, 2, :],
                                     gg[s][g // 8][:, g % 8, 0, :], start=False, stop=True)
                # slots (0,2,1,3) hold (g0 Yr, g0 Yi, g1 Yr, g1 Yi)
                copy(yy[s][:, 2 * gp:2 * gp + 2, :, :],
                     pe[:].rearrange("p (a b) n -> p b a n", a=2))
            # Nyquist einsum lands in the (f=0,g=0) rows of the imag half
            # (the otherwise meaningless Zi[0] slots); regroup2 then routes
            # it to yh[.,1,...] row 0, where dmat row 0 of D1 applies it.
            pyn = ps.tile([KIN, 256], F32, tag="ps")
            nc.tensor.matmul(pyn[:], wnyq[:], gnyq[s][:],
                             start=True, stop=True)
            copy(yy[s][0:KIN, 0, 1, :], pyn[:])

        def regroup2(s, r0):
            # yh[s][(f,g), i, ri, b] = yy[s][(f,i), g, ri, b]; one DMA per i
            for i in range(KOUT):
                RR[(r0 + i) % 16].dma_start(
                    out=yh[s][i // 8][:, i % 8, :, :],
                    in_=yy[s][i::16, :, :, :])

        def stage_I(s):
            # inverse DFT with the data stationary -> [b, m] orientation;
            # two i per PSUM slot: i even -> slots 0 (c=0) / 2 (c=1),
            # i odd -> slots 1 / 3
            for ig in range(4):
                osb = osp.tile([128, 2, 1024], F32, tag="os")
                for ip in range(2):
                    po = ps.tile([128, 4, 256], F32, tag="ps")
                    for half in range(2):
                        i = 4 * ig + 2 * ip + half
                        for c in range(2):   # bs chunk -> banks 0 / 1
                            nc.tensor.matmul(
                                po[:, 2 * c + half, :],
                                yh[s][i // 8][:, i % 8, 0,
                                              128 * c:128 * (c + 1)],
                                dmat[:, 0, :], start=True, stop=False)
                        for c in range(2):
                            nc.tensor.matmul(
                                po[:, 2 * c + half, :],
                                yh[s][i // 8][:, i % 8, 1,
                                              128 * c:128 * (c + 1)],
                                dmat[:, 1, :], start=False, stop=True)
                    # slots (0,1,2,3) = (i0c0, i1c0, i0c1, i1c1) = dst order
                    copy(osb[:, :, 512 * ip:512 * (ip + 1)], po[:])
                # store [256 rows, 1024 cols] of out
                eng = nc.sync if s == 0 else nc.scalar
                eng.dma_start(
                    out_d[SLAB * s:SLAB * (s + 1),
                          1024 * ig:1024 * (ig + 1)].rearrange(
                              "(c p) m -> p c m", c=2),
                    osb[:],
                )

        # ---- pipelined emission (per-engine queues in execution order)
        stage_T(0, 0)
        stage_T(0, 1)
        stage_F(0)
        regroup1(0, 1)
        nc.gpsimd.dma_start(out=gnyq[0][:], in_=xf[0][0:1, :, 1, :])
        stage_T(1, 0)
        stage_T(1, 1)
        stage_F(1)
        regroup1(1, 0)
        nc.gpsimd.dma_start(out=gnyq[1][:], in_=xf[1][0:1, :, 1, :])
        stage_E(0)
        regroup2(0, 2)
        stage_E(1)
        regroup2(1, 0)
        stage_I(0)
        stage_I(1)

    nc.compile()
    return nc


def _get_program():
    if "nc" not in _CACHE:
        _CACHE["nc"] = _build_program()
    return _CACHE["nc"]


def _install_ntff_hook():
    """Provide antenv.axon_hooks (absent in this image) so that
    run_bass_kernel_spmd(trace=True) can capture NTFF profiles through the
    axon client library."""
    import sys
    import types
    import ctypes
    import contextlib

    if "antenv.axon_hooks" in sys.modules:
        return
    try:
        lib = ctypes.CDLL("/opt/axon/libaxon_pjrt.so")
    except OSError:
        return
    if not hasattr(lib, "axon_start_nrt_profile"):
        return
    lib.axon_start_nrt_profile.argtypes = [
        ctypes.POINTER(ctypes.c_int64),
        ctypes.c_size_t,
    ]
    lib.axon_start_nrt_profile.restype = ctypes.c_int64
    lib.axon_stop_nrt_profile.argtypes = [ctypes.c_char_p]
    lib.axon_stop_nrt_profile.restype = ctypes.c_int64

    @contextlib.contextmanager
    def _hook(output_dir, device_ids):
        import jax

        jax.devices()
        if device_ids:
            ids = (ctypes.c_int64 * len(device_ids))(*device_ids)
            rc = lib.axon_start_nrt_profile(ids, len(device_ids))
        else:
            rc = lib.axon_start_nrt_profile(None, 0)
        if rc != 0:
            raise RuntimeError(f"axon_start_nrt_profile rc={rc}")
        try:
            yield
        finally:
            n = lib.axon_stop_nrt_profile(str(output_dir).encode())
            print(f"ntff profile: {n} file(s) -> {output_dir}")

    mod = types.ModuleType("antenv.axon_hooks")
    state = {"hook": _hook}
    mod.get_axon_ntff_profile_hook = lambda: state["hook"]
    mod.set_axon_ntff_profile_hook = lambda h: state.update(hook=h)
    sys.modules["antenv.axon_hooks"] = mod
    import antenv

    antenv.axon_hooks = mod


def kernel(x, W_real, W_imag, block_size, out_features):
    global LAST_RESULTS
    x = np.ascontiguousarray(np.asarray(x, dtype=np.float32))
    Wr = np.asarray(W_real, dtype=np.float32)
    Wi = np.asarray(W_imag, dtype=np.float32)
    assert int(block_size) == BS and int(out_features) == D_OUT
    assert x.shape == (B_FULL, D_IN) and Wr.shape == (KOUT, KIN, 129)

    nc = _get_program()
    consts = _build_consts(Wr, Wi)
    core_ids = list(range(NCORES))
    in_maps = [
        {"x": np.ascontiguousarray(x[c * BC:(c + 1) * BC]), **consts}
        for c in core_ids
    ]
    trace = bool(int(os.environ.get("KERNEL_TRACE", "0")))
    if trace:
        _install_ntff_hook()
    res = run_bass_kernel_spmd(nc, in_maps, core_ids, trace=trace)
    LAST_RESULTS = res
    out = np.concatenate([res.results[c]["out"] for c in core_ids], axis=0)
    return np.ascontiguousarray(out.astype(np.float32))


# revision 24
# speedup vs baseline: 1.4927x; 1.1799x over previous
"""Trainium2 Bass kernel for BlockFFTDirectPrior.

Computes out = irfft(einsum('bjn,ijn->bin', rfft(x_blocks), conj(W)))
reshaped to [B, 4096], for x [4096, 4096] f32, W [16, 16, 129] complex
(block size 256).

Strategy: data-parallel over the batch axis across 8 NeuronCores (512 rows
each). Per core, the 512 rows are processed as two 256-row slabs flowing
through a 4-stage PE pipeline so input DMA, compute, the two partition
regroups, and output stores all overlap:

  T: transpose x tiles (PE transpose vs identity)     -> xt [t, dc, b] fp16
  F: real DFT as fp16 matmuls (contract t)            -> xf [n, j, ri, b]
  E: per-frequency 16x16 complex mixing as 8-frequency
     block-diagonal fp16 matmuls (K = (f,j) = 128)    -> yy [n', g, ri, b]
  I: real inverse DFT, data stationary (fp16 weights,
     FWL), which restores [b, m] orientation for free -> out rows

DFT/IDFT row order is swizzled to r = f*16+g so the two partition
regroups between F/E and E/I become per-g (resp. per-i) affine
SBUF->SBUF DMAs that carry both the real and imag halves in one
transfer. Regroups and stores ride the two fast HWDGE rings (sync +
scalar) ordered to match the pipeline; intermediates are fp16, which
halves regroup bytes and doubles LDWEIGHTS rate (FWL). Accumulation
groups are bank-interleaved in PSUM so one LDWEIGHTS feeds two matmuls.
"""

import os
import numpy as np
from contextlib import ExitStack

import concourse.bass as bass
import concourse.tile as tile
from concourse import bacc, mybir
from concourse.bass_utils import run_bass_kernel_spmd

NCORES = 8
B_FULL, D_IN, D_OUT, BS = 4096, 4096, 4096, 256
BC = B_FULL // NCORES          # 512 batch rows per core
SLAB = 256                     # rows per pipeline slab (2 slabs per core)
KIN = KOUT = 16
NG = 16                        # groups of 8 frequencies covering n=0..127
F16 = mybir.dt.float16
F32 = mybir.dt.float32

_CACHE = {}
LAST_RESULTS = None            # BassKernelResults of the most recent run


# DFT row swizzle: row r holds frequency n = 16*((r%32)//4) + 4*(r//32) +
# (r%4).  Group g = n%16 then occupies rows {32*(g//4) + g%4 + 4k}, a
# stride-4 partition slice: its 8 partitions map to 8 distinct SBUF AXI
# ports (port = 2*((p%32)//4) + p//64), twice the read bandwidth of a
# stride-16 pattern.  The same stride-4 property holds for the E-output
# rows of each i, and the regrouped yh rows come out in natural frequency
# order (row p holds n = p), so the IDFT matrix needs no permutation.
PERM = np.array([16 * ((r % 32) // 4) + 4 * (r // 32) + (r % 4)
                 for r in range(128)])


def _grp_rows(g):
    # first row of the stride-4 slice holding group/output index g
    return 32 * (g // 4) + (g % 4)


def _build_consts(W_real, W_imag):
    """Constant matrices in the exact SBUF layouts the kernel reads."""
    f16 = np.float16
    t = np.arange(BS)
    n0 = np.arange(128)
    ang = 2.0 * np.pi / BS

    CF0 = np.cos(ang * np.outer(t, n0))
    CF1 = np.empty((BS, 128))
    CF1[:, 0] = np.cos(np.pi * t)
    p = np.arange(1, 128)
    CF1[:, 1:] = -np.sin(ang * np.outer(t, p))
    CF0 = CF0[:, PERM]
    CF1 = CF1[:, PERM]
    cfs = np.stack([
        np.concatenate([CF0[:128], CF0[128:]], axis=1),
        np.concatenate([CF1[:128], CF1[128:]], axis=1),
    ], axis=1).astype(f16)                                  # [128, 2, 256]

    # wpk[(k*16+j), g, c, r'(i,k)] = M_c[i, j, 16k+g];  M = (Wr, Wi, -Wi);
    # r'(i,k) = 32*(i//4) + i%4 + 4k is the E-output row for (i, k)
    wpk = np.zeros((128, NG, 3, 128), dtype=f16)
    jj = np.arange(KIN)[:, None, None]
    ii = np.arange(KOUT)[None, :, None]
    kk = np.arange(8)[None, None, :]
    rr = 32 * (ii // 4) + ii % 4 + 4 * kk
    for g in range(NG):
        for c, M in enumerate((W_real, W_imag, -W_imag)):
            wpk[kk * 16 + jj, g, c, rr] = M[ii, jj, 16 * kk + g]
    # wnyq[j, r'(i,0)] = Wr[i, j, 128]: the Nyquist matmul output lands
    # directly on the E-output rows for k=0, 32-aligned for the copies
    wnyq = np.zeros((KIN, 128), dtype=f16)
    for i in range(KOUT):
        wnyq[:, 32 * (i // 4) + i % 4] = W_real[i, :, 128]

    # IDFT matrices in natural frequency row order (yh row p holds n = p)
    m = np.arange(BS)
    D0 = np.empty((128, BS))
    D0[0] = 1.0 / BS
    nn = np.arange(1, 128)
    D0[1:] = (2.0 / BS) * np.cos(ang * np.outer(nn, m))
    D1 = np.empty((128, BS))
    D1[0] = ((-1.0) ** m) / BS
    D1[1:] = -(2.0 / BS) * np.sin(ang * np.outer(nn, m))
    dmat = np.stack([D0, D1], axis=1).astype(f16)  # [128, 2, 256]

    ident = np.eye(128, dtype=np.float32)
    return {"cfs": cfs, "wpk": wpk, "wnyq": wnyq, "dmat": dmat, "ident": ident}


def _build_program():
    nc = bacc.Bacc(
        "TRN2", target_bir_lowering=False, debug=False, num_devices=NCORES
    )
    x_d = nc.dram_tensor("x", [BC, D_IN], F32, kind="ExternalInput").ap()
    cfs_d = nc.dram_tensor("cfs", [128, 2, 256], F16, kind="ExternalInput").ap()
    wpk_d = nc.dram_tensor("wpk", [128, NG, 3, 128], F16, kind="ExternalInput").ap()
    wnyq_d = nc.dram_tensor("wnyq", [KIN, 128], F16, kind="ExternalInput").ap()
    dmat_d = nc.dram_tensor("dmat", [128, 2, 256], F16, kind="ExternalInput").ap()
    ident_d = nc.dram_tensor("ident", [128, 128], F32, kind="ExternalInput").ap()
    out_d = nc.dram_tensor("out", [BC, D_OUT], F32, kind="ExternalOutput").ap()

    cp_state = [0]

    with tile.TileContext(nc) as tc, ExitStack() as ctx:
        def copy(dst, src):
            # alternate PSUM->SBUF copies between DVE and ACT
            if cp_state[0] % 2 == 0:
                nc.vector.tensor_copy(dst, src)
            else:
                nc.scalar.copy(dst, src)
            cp_state[0] += 1

        consts = ctx.enter_context(tc.tile_pool(name="consts", bufs=1))
        xsp = ctx.enter_context(tc.tile_pool(name="xsp", bufs=3))
        xtp = ctx.enter_context(tc.tile_pool(name="xtp", bufs=1))
        mid1 = ctx.enter_context(tc.tile_pool(name="mid1", bufs=1))
        mid2 = ctx.enter_context(tc.tile_pool(name="mid2", bufs=2))
        osp = ctx.enter_context(tc.tile_pool(name="osp", bufs=2))
        ps = ctx.enter_context(tc.tile_pool(name="ps", bufs=4, space="PSUM"))

        cfs = consts.tile([128, 2, 256], F16)
        wpk = consts.tile([128, NG, 3, 128], F16)
        wnyq = consts.tile([KIN, 128], F16)
        dmat = consts.tile([128, 2, 256], F16)
        ident = consts.tile([128, 128], F32)

        # ident/cfs (small, needed first) ride the otherwise-idle scalar
        # ring; bulky-but-late consts ride the gpsimd (SWDGE) ring
        nc.scalar.dma_start(ident[:], ident_d)
        nc.scalar.dma_start(cfs[:], cfs_d)
        nc.gpsimd.dma_start(wpk[:], wpk_d)
        nc.gpsimd.dma_start(dmat[:], dmat_d)
        nc.gpsimd.dma_start(wnyq[:], wnyq_d)

        # ---- input loads: all on the sync ring, in order.  One ring's
        # engines drain its DMAs in issue order, so chunk 0 completes
        # first (~7us) instead of fair-sharing with later chunks.
        xs = [xsp.tile([128, D_IN], F32, tag="xs", name=f"xs{i}")
              for i in range(4)]
        for bc in range(4):
            for h in range(2):
                nc.sync.dma_start(
                    xs[bc][:, 2048 * h:2048 * (h + 1)],
                    x_d[128 * bc:128 * (bc + 1), 2048 * h:2048 * (h + 1)])

        # per-slab tiles; gg/yh double-buffered so regroup DMAs of slab 1
        # don't wait on slab 0's consumers
        xt = [xtp.tile([128, 32, SLAB], F16, tag="xt", name=f"xt{i}")
              for i in range(2)]
        xf = [mid1.tile([128, KIN, 2, SLAB], F16, tag="xf", name=f"xf{i}")
              for i in range(2)]
        # gg/yh are split into halves so E/I can start once the first half
        # of a regroup has landed instead of waiting for all 16 DMAs
        gg = [[mid2.tile([128, NG // 2, 2, SLAB], F16, tag=f"gg{h}",
                         name=f"gg{i}_{h}") for h in range(2)]
              for i in range(2)]
        yy = [mid1.tile([128, NG, 2, SLAB], F16, tag="yy", name=f"yy{i}")
              for i in range(2)]
        yh = [[mid2.tile([128, KOUT // 2, 2, SLAB], F16, tag=f"yh{h}",
                         name=f"yh{i}_{h}") for h in range(2)]
              for i in range(2)]
        gnyq = [consts.tile([KIN, SLAB], F16, tag="gnyq", name=f"gnyq{i}")
                for i in range(2)]

        # regroup ring plans: regroup1(s0) avoids sync (still loading x);
        # later regroups spread across all three DGE units
        RR_NOSYNC = [nc.gpsimd, nc.scalar] * 8
        RR_ALL = [nc.gpsimd, nc.sync, nc.gpsimd, nc.scalar] * 4

        def stage_T(s, c):
            # transpose chunk bc=2s+c of x into xt[s][:, :, 128c:128c+128]
            bc = 2 * s + c
            for dcg in range(4):
                pt = ps.tile([128, 8, 128], F32, tag="ps")
                for q in range(8):
                    dc = dcg * 8 + q
                    nc.tensor.transpose(
                        pt[:, q, :], xs[bc][:, 128 * dc:128 * (dc + 1)],
                        ident[:],
                    )
                copy(xt[s][:, 8 * dcg:8 * dcg + 8, 128 * c:128 * (c + 1)],
                     pt[:])

        def stage_F(s):
            # real DFT: xf[s][n, j, which, b] = sum_t cfs[t, which, n] xt[t, (j,tc), b]
            # j pairs map to the two banks of one PSUM slot (slots 0 / 2) so
            # accumulation groups never interleave within a bank, while each
            # LDWEIGHTS (cfs half) feeds two matmuls.
            # slot map: j0->0(bank0), j1->2(bank1), then j2->1(bank0), j3->3
            # (a bank's second group starts only after its first stopped)
            for which in range(2):
                for jg in range(4):
                    pf = ps.tile([128, 4, 256], F32, tag="ps")
                    for half in range(2):
                        for tc_ in range(2):
                            for bank in range(2):
                                j = 4 * jg + 2 * half + bank
                                nc.tensor.matmul(
                                    pf[:, 2 * bank + half, :],
                                    cfs[:, which, 128 * tc_:128 * (tc_ + 1)],
                                    xt[s][:, 2 * j + tc_, :],
                                    start=(tc_ == 0),
                                    stop=(tc_ == 1),
                                )
                    # slots (0,2,1,3) hold j order (0,1,2,3)
                    copy(xf[s][:, 4 * jg:4 * jg + 4, which, :],
                         pf[:].rearrange("p (a b) n -> p b a n", a=2))

        def regroup1(s, rings):
            # gg[s][(k,j), g, ri, b] = xf[s][r(k,g), j, ri, b]; one DMA per
            # g reading a stride-4 partition slice (8 SBUF ports)
            for g in range(NG):
                a = _grp_rows(g)
                rings[g % len(rings)].dma_start(
                    out=gg[s][g // 8][:, g % 8, :, :],
                    in_=xf[s][a:a + 29:4, :, :, :])

        def stage_E(s):
            # per-frequency-group complex mixing, two g per PSUM slot:
            # g even -> slots 0 (Yr) / 2 (Yi), g odd -> slots 1 / 3
            for gp in range(8):
                pe = ps.tile([128, 4, 256], F32, tag="ps")
                for half in range(2):
                    g = 2 * gp + half
                    nc.tensor.matmul(pe[:, half, :], wpk[:, g, 0, :],
                                     gg[s][g // 8][:, g % 8, 0, :], start=True, stop=False)
                    nc.tensor.matmul(pe[:, 2 + half, :], wpk[:, g, 0, :],
                                     gg[s][g // 8][:, g % 8, 1, :], start=True, stop=False)
                    nc.tensor.matmul(pe[:, half, :], wpk[:, g, 1, :],
                                     gg[s][g // 8][:, g % 8, 1, :], start=False, stop=True)
                    nc.tensor.matmul(pe[:, 2 + half, :], wpk[:, g, 2, :],
                                     gg[s][g // 8][:, g % 8, 0, :], start=False, stop=True)
                # slots (0,2,1,3) hold (g0 Yr, g0 Yi, g1 Yr, g1 Yi)
                copy(yy[s][:, 2 * gp:2 * gp + 2, :, :],
                     pe[:].rearrange("p (a b) n -> p b a n", a=2))
            # Nyquist einsum lands in the (f=0,g=0) rows of the imag half
            # (the otherwise meaningless Zi[0] slots); regroup2 then routes
            # it to yh[.,1,...] row 0, where dmat row 0 of D1 applies it.
            pyn = ps.tile([128, 256], F32, tag="ps")
            nc.tensor.matmul(pyn[:], wnyq[:], gnyq[s][:],
                             start=True, stop=True)
            # Zi[0] rows for i = 4a+c sit at partition 32a+c; pyn rows
            # match, so each copy reads/writes a 32-aligned partition base
            for a in range(4):
                copy(yy[s][32 * a:32 * a + 4, 0, 1, :],
                     pyn[32 * a:32 * a + 4, :])

        def regroup2(s, rings):
            # yh[s][n, i, ri, b] = yy[s][r'(i,k), g, ri, b]; one DMA per i
            # reading a stride-4 partition slice; yh rows come out in
            # natural frequency order n = 16k+g
            for i in range(KOUT):
                a = _grp_rows(i)
                rings[i % len(rings)].dma_start(
                    out=yh[s][i // 8][:, i % 8, :, :],
                    in_=yy[s][a:a + 29:4, :, :, :])

        def stage_I(s):
            # inverse DFT with the data stationary -> [b, m] orientation;
            # two i per PSUM slot: i even -> slots 0 (c=0) / 2 (c=1),
            # i odd -> slots 1 / 3
            for ig in range(4):
                osb = osp.tile([128, 2, 1024], F32, tag="os")
                for ip in range(2):
                    po = ps.tile([128, 4, 256], F32, tag="ps")
                    for half in range(2):
                        i = 4 * ig + 2 * ip + half
                        for c in range(2):   # bs chunk -> banks 0 / 1
                            nc.tensor.matmul(
                                po[:, 2 * c + half, :],
                                yh[s][i // 8][:, i % 8, 0,
                                              128 * c:128 * (c + 1)],
                                dmat[:, 0, :], start=True, stop=False)
                        for c in range(2):
                            nc.tensor.matmul(
                                po[:, 2 * c + half, :],
                                yh[s][i // 8][:, i % 8, 1,
                                              128 * c:128 * (c + 1)],
                                dmat[:, 1, :], start=False, stop=True)
                    # slots (0,1,2,3) = (i0c0, i1c0, i0c1, i1c1) = dst order
                    copy(osb[:, :, 512 * ip:512 * (ip + 1)], po[:])
                # store [256 rows, 1024 cols] of out
                eng = nc.sync if s == 0 else nc.scalar
                eng.dma_start(
                    out_d[SLAB * s:SLAB * (s + 1),
                          1024 * ig:1024 * (ig + 1)].rearrange(
                              "(c p) m -> p c m", c=2),
                    osb[:],
                )

        # ---- pipelined emission (per-engine queues in execution order)
        # PE warmup: dummy transposes of ident bridge the input-load
        # window so HAM unthrottles before the real work starts
        for w in range(4):
            pw = ps.tile([128, 4, 128], F32, tag="ps", name=f"pw{w}")
            for q in range(4):
                nc.tensor.transpose(pw[:, q, :], ident[:], ident[:])

        stage_T(0, 0)
        stage_T(0, 1)
        stage_F(0)
        regroup1(0, RR_NOSYNC)
        nc.gpsimd.dma_start(out=gnyq[0][:], in_=xf[0][0:1, :, 1, :])
        stage_T(1, 0)
        stage_T(1, 1)
        stage_F(1)
        regroup1(1, RR_ALL)
        nc.gpsimd.dma_start(out=gnyq[1][:], in_=xf[1][0:1, :, 1, :])
        stage_E(0)
        regroup2(0, RR_ALL)
        stage_E(1)
        regroup2(1, RR_ALL)
        stage_I(0)
        stage_I(1)

    nc.compile()
    return nc


def _get_program():
    if "nc" not in _CACHE:
        _CACHE["nc"] = _build_program()
    return _CACHE["nc"]


def _install_ntff_hook():
    """Provide antenv.axon_hooks (absent in this image) so that
    run_bass_kernel_spmd(trace=True) can capture NTFF profiles through the
    axon client library."""
    import sys
    import types
    import ctypes
    import contextlib

    if "antenv.axon_hooks" in sys.modules:
        return
    try:
        lib = ctypes.CDLL("/opt/axon/libaxon_pjrt.so")
    except OSError:
        return
    if not hasattr(lib, "axon_start_nrt_profile"):
        return
    lib.axon_start_nrt_profile.argtypes = [
        ctypes.POINTER(ctypes.c_int64),
        ctypes.c_size_t,
    ]
    lib.axon_start_nrt_profile.restype = ctypes.c_int64
    lib.axon_stop_nrt_profile.argtypes = [ctypes.c_char_p]
    lib.axon_stop_nrt_profile.restype = ctypes.c_int64

    @contextlib.contextmanager
    def _hook(output_dir, device_ids):
        import jax

        jax.devices()
        if device_ids:
            ids = (ctypes.c_int64 * len(device_ids))(*device_ids)
            rc = lib.axon_start_nrt_profile(ids, len(device_ids))
        else:
            rc = lib.axon_start_nrt_profile(None, 0)
        if rc != 0:
            raise RuntimeError(f"axon_start_nrt_profile rc={rc}")
        try:
            yield
        finally:
            n = lib.axon_stop_nrt_profile(str(output_dir).encode())
            print(f"ntff profile: {n} file(s) -> {output_dir}")

    mod = types.ModuleType("antenv.axon_hooks")
    state = {"hook": _hook}
    mod.get_axon_ntff_profile_hook = lambda: state["hook"]
    mod.set_axon_ntff_profile_hook = lambda h: state.update(hook=h)
    sys.modules["antenv.axon_hooks"] = mod
    import antenv

    antenv.axon_hooks = mod


def kernel(x, W_real, W_imag, block_size, out_features):
    global LAST_RESULTS
    x = np.ascontiguousarray(np.asarray(x, dtype=np.float32))
    Wr = np.asarray(W_real, dtype=np.float32)
    Wi = np.asarray(W_imag, dtype=np.float32)
    assert int(block_size) == BS and int(out_features) == D_OUT
    assert x.shape == (B_FULL, D_IN) and Wr.shape == (KOUT, KIN, 129)

    nc = _get_program()
    consts = _build_consts(Wr, Wi)
    core_ids = list(range(NCORES))
    in_maps = [
        {"x": np.ascontiguousarray(x[c * BC:(c + 1) * BC]), **consts}
        for c in core_ids
    ]
    trace = bool(int(os.environ.get("KERNEL_TRACE", "0")))
    if trace:
        _install_ntff_hook()
    res = run_bass_kernel_spmd(nc, in_maps, core_ids, trace=trace)
    LAST_RESULTS = res
    out = np.concatenate([res.results[c]["out"] for c in core_ids], axis=0)
    return np.ascontiguousarray(out.astype(np.float32))
